# revision 1
# baseline (speedup 1.0000x reference)
"""Trainium2 Bass kernel for DescriptorMatcher (mutual nearest neighbor matching).

Problem: given desc0 [B,N,D], desc1 [B,M,D] (B=4, N=M=8192, D=128, fp32):
    sim     = desc0 @ desc1^T                      [B,N,M]
    score0  = max_m sim                            [B,N]
    match01 = argmax_m sim                         [B,N]
    match10 = argmax_n sim                         [B,M]
    valid   = (match10[match01[n]] == n) & (score0 > 0.1)
returns (match01, score0, valid).

Key reformulation: the mutual check never needs match10 indices:
    match10[match01[n]] == n  <=>  score0[n] == colmax[match01[n]]
(exact fp32 equality is safe: both sides are max-chains over the same
on-device fp32 values, and max is exact).

Sharding: 8 cores = 4 batches x 2 row-halves (4096 rows each).

Two-phase execution (VectorE is the bottleneck; this needs 2 full VectorE
passes per element instead of 3):

Phase 1 (per core, ~590 us): for each of 32 n-tiles:
    PE:  16 fp32 matmuls -> PSUM [128,2048] x4
    ACT: copy PSUM -> SBUF row buffer [128, 8192]
    DVE: colacc = max(colacc, row)                      (column side)
    DVE: one fused reduce [128,8,1024]-view -> CM[t]    (8 chunk maxima/row)
  then partition_all_reduce(max) -> partial colmax [8192].
  Host: score0 = CM.max(1); c* = CM.argmax(1) (first occurrence); groups
  rows by winning chunk.

Phase 2 (per core, ~125 us): for each group (rows sharing winning chunk c),
  recompute sim[:, c*1024:(c+1)*1024] with identically-laid-out fp32 matmuls
  (bit-exact: each PE output element depends only on its own lhsT/rhs column
  pair), then max_index(score, chunk) gives the exact first-occurrence
  within-chunk position. match01 = c*1024 + within.

Rows overflowing a group's padded capacity (impossible for anything
gaussian-like; needs >768 of 4096 rows sharing one winning chunk) fall back
to a host-side recompute of that row.
"""

import numpy as np

import concourse.bass as bass  # noqa: F401  (bass must import before tile)
import concourse.mybir as mybir
import concourse.tile as tile
from concourse import bacc, bass_isa

B, N, M, D = 4, 8192, 8192, 128
NCORES = 8
HALF = N // 2          # rows per core
NT = HALF // 128       # 32 n-tiles per core
CW = 1024              # input-DMA chunk width
PAD = 640              # phase-2 rows per chunk-group (+6 sigma of binomial;
                       # overflow degrades to host fallback, never wrong)
NST = 8 * PAD // 128   # 48 phase-2 sub-tiles


def _build1():
    f32 = mybir.dt.float32
    nc = bacc.Bacc("TRN2", target_bir_lowering=False, debug=False,
                   num_devices=NCORES)
    at = nc.dram_tensor("at", [D, HALF], f32, kind="ExternalInput").ap()
    bt = nc.dram_tensor("bt", [D, M], f32, kind="ExternalInput").ap()
    cm_o = nc.dram_tensor("cm", [128, NT * 8], f32, kind="ExternalOutput").ap()
    colp_o = nc.dram_tensor("colp", [1, M], f32, kind="ExternalOutput").ap()

    with tile.TileContext(nc) as tc:
        with tc.tile_pool(name="big", bufs=1) as big, \
             tc.tile_pool(name="rows", bufs=3) as rows, \
             tc.tile_pool(name="ps", bufs=2, space="PSUM") as ps:
            atb = big.tile([128, HALF], f32, name="atb")
            btb = big.tile([128, M], f32, name="btb")
            for c in range(0, HALF, CW):
                nc.sync.dma_start(atb[:, c:c + CW], at[:, c:c + CW])
            for c in range(0, M, CW):
                nc.sync.dma_start(btb[:, c:c + CW], bt[:, c:c + CW])
            colacc = big.tile([128, M], f32, name="colacc")
            cm_all = big.tile([128, NT * 8], f32, name="cm_all")
            for t in range(NT):
                row = rows.tile([128, M], f32, tag="row", name="row")
                for c in range(4):
                    pt = ps.tile([128, 2048], f32, tag="pt", name="pt")
                    for j in range(4):
                        mlo = c * 2048 + j * 512
                        nc.tensor.matmul(pt[:, j * 512:(j + 1) * 512],
                                         atb[:, t * 128:(t + 1) * 128],
                                         btb[:, mlo:mlo + 512],
                                         start=True, stop=True)
                    nc.scalar.copy(row[:, c * 2048:(c + 1) * 2048], pt[:])
                if t == 0:
                    for c in range(4):
                        nc.vector.tensor_copy(colacc[:, c * 2048:(c + 1) * 2048],
                                              row[:, c * 2048:(c + 1) * 2048])
                else:
                    nc.vector.tensor_tensor(colacc[:], colacc[:], row[:],
                                            op=mybir.AluOpType.max)
                v = row[:].rearrange("p (c w) -> p c w", w=1024)
                nc.vector.tensor_reduce(cm_all[:, t * 8:(t + 1) * 8], v,
                                        axis=mybir.AxisListType.X,
                                        op=mybir.AluOpType.max)
            cp = rows.tile([128, M], f32, tag="row", name="cp")
            nc.gpsimd.partition_all_reduce(cp[:], colacc[:], channels=128,
                                           reduce_op=bass_isa.ReduceOp.max)
            nc.sync.dma_start(cm_o[:], cm_all[:])
            nc.sync.dma_start(colp_o[:], cp[0:1, :])
    nc.compile()
    return nc


def _build2():
    f32, u32 = mybir.dt.float32, mybir.dt.uint32
    nc = bacc.Bacc("TRN2", target_bir_lowering=False, debug=False,
                   num_devices=NCORES)
    at2 = nc.dram_tensor("at2", [D, 8 * PAD], f32, kind="ExternalInput").ap()
    bt = nc.dram_tensor("bt", [D, M], f32, kind="ExternalInput").ap()
    sg = nc.dram_tensor("sg", [128, NST], f32, kind="ExternalInput").ap()
    idx_o = nc.dram_tensor("idx", [128, NST * 8], u32, kind="ExternalOutput").ap()
    with tile.TileContext(nc) as tc:
        with tc.tile_pool(name="big", bufs=1) as big, \
             tc.tile_pool(name="work", bufs=3) as work, \
             tc.tile_pool(name="ps", bufs=4, space="PSUM") as ps:
            a2b = big.tile([128, 8 * PAD], f32, name="a2b")
            btb = big.tile([128, M], f32, name="btb")
            sgb = big.tile([128, NST], f32, name="sgb")
            nc.sync.dma_start(sgb[:], sg[:])
            # chunked so group 0's matmuls start before all input has landed
            for c in range(0, 8 * PAD, PAD):
                nc.sync.dma_start(a2b[:, c:c + PAD], at2[:, c:c + PAD])
            for c in range(0, M, CW):
                nc.sync.dma_start(btb[:, c:c + CW], bt[:, c:c + CW])
            idx8 = big.tile([128, NST * 8], u32, name="idx8")
            KP = PAD // 128
            for g in range(8):
                for k in range(KP):
                    st = g * KP + k
                    pt = ps.tile([128, 1024], f32, tag="pt", name="pt")
                    for j in range(2):
                        nc.tensor.matmul(
                            pt[:, j * 512:(j + 1) * 512],
                            a2b[:, st * 128:(st + 1) * 128],
                            btb[:, g * 1024 + j * 512: g * 1024 + (j + 1) * 512],
                            start=True, stop=True)
                    ch = work.tile([128, 1024], f32, tag="ch", name="ch")
                    nc.scalar.copy(ch[:], pt[:])
                    sc8 = work.tile([128, 8], f32, tag="sc8", name="sc8")
                    nc.vector.tensor_copy(
                        sc8[:], sgb[:, st:st + 1].to_broadcast((128, 8)))
                    nc.vector.max_index(idx8[:, st * 8:(st + 1) * 8],
                                        sc8[:], ch[:])
            nc.sync.dma_start(idx_o[:], idx8[:])
    nc.compile()
    return nc


_cached = None


def _make_exec(nc):
    import jax
    from jax.sharding import Mesh, PartitionSpec
    from jax.experimental.shard_map import shard_map
    from concourse import bass2jax
    from concourse.bass2jax import _bass_exec_p

    partition_name = nc.partition_id_tensor.name if nc.partition_id_tensor else None
    in_names, out_names, out_avals, out_shapes = [], [], [], []
    for alloc in nc.m.functions[0].allocations:
        if not isinstance(alloc, mybir.MemoryLocationSet):
            continue
        name = alloc.memorylocations[0].name
        if alloc.kind == "ExternalInput":
            if name != partition_name:
                in_names.append(name)
        elif alloc.kind == "ExternalOutput":
            shape = tuple(alloc.tensor_shape)
            dtype = mybir.dt.np(alloc.dtype)
            out_names.append(name)
            out_shapes.append((shape, dtype))
            out_avals.append(jax.core.ShapedArray(shape, dtype))
    n_params = len(in_names)
    n_outs = len(out_names)
    all_in_names = in_names + out_names
    if partition_name is not None:
        all_in_names = all_in_names + [partition_name]

    def _body(*args):
        operands = list(args)
        if partition_name is not None:
            operands.append(bass2jax.partition_id_tensor())
        outs = _bass_exec_p.bind(
            *operands, out_avals=tuple(out_avals), in_names=tuple(all_in_names),
            out_names=tuple(out_names), lowering_input_output_aliases=(),
            sim_require_finite=True, sim_require_nnan=True, nc=nc)
        return tuple(outs)

    devices = jax.devices()[:NCORES]
    mesh = Mesh(np.asarray(devices), ("core",))
    in_specs = (PartitionSpec("core"),) * (n_params + n_outs)
    out_specs = (PartitionSpec("core"),) * n_outs
    fn = jax.jit(shard_map(_body, mesh=mesh, in_specs=in_specs,
                           out_specs=out_specs, check_rep=False),
                 keep_unused=True)
    return {"fn": fn, "in_names": in_names, "out_names": out_names,
            "out_shapes": out_shapes, "nc": nc}


def _run(ex, ins):
    """ins: dict name -> [NCORES, *shape]; returns dict name -> [NCORES, *shape]."""
    concat_in = [np.ascontiguousarray(ins[n].reshape(-1, *ins[n].shape[2:]))
                 for n in ex["in_names"]]
    concat_zeros = [np.zeros((NCORES * s[0], *s[1:]), dt)
                    for (s, dt) in ex["out_shapes"]]
    out_arrs = ex["fn"](*concat_in, *concat_zeros)
    return {name: np.asarray(out_arrs[i]).reshape(NCORES, *ex["out_shapes"][i][0])
            for i, name in enumerate(ex["out_names"])}


def kernel(desc0, desc1):
    global _cached
    desc0 = np.asarray(desc0, dtype=np.float32)
    desc1 = np.asarray(desc1, dtype=np.float32)
    assert desc0.shape == (B, N, D) and desc1.shape == (B, M, D)

    if _cached is None:
        _cached = (_make_exec(_build1()), _make_exec(_build2()))
    ex1, ex2 = _cached

    a_slab = np.stack([desc0[b, h * HALF:(h + 1) * HALF]
                       for b in range(B) for h in range(2)])      # [8,4096,128]
    bt_all = np.stack([desc1[b].transpose(1, 0)
                       for b in range(B) for h in range(2)])      # [8,128,8192]
    at_all = a_slab.transpose(0, 2, 1)                            # [8,128,4096]

    r1 = _run(ex1, {"at": at_all, "bt": bt_all})

    # host glue: score/chunk-argmax + grouping for phase 2
    cm = r1["cm"].reshape(NCORES, 128, NT, 8).transpose(0, 2, 1, 3) \
                 .reshape(NCORES, HALF, 8)
    score0_c = cm.max(axis=2)                                     # [8, 4096]
    cstar_c = cm.argmax(axis=2)                                   # [8, 4096]

    at2 = np.zeros((NCORES, D, 8 * PAD), np.float32)
    sg = np.full((NCORES, 128, NST), 1e30, np.float32)
    slot_of_row = np.full((NCORES, HALF), -1, np.int64)
    overflow = []                                                 # (core, row)
    for core in range(NCORES):
        for g in range(8):
            rows = np.nonzero(cstar_c[core] == g)[0]
            if len(rows) > PAD:
                overflow.extend((core, r) for r in rows[PAD:])
                rows = rows[:PAD]
            slots = g * PAD + np.arange(len(rows))
            slot_of_row[core, rows] = slots
            at2[core][:, slots] = a_slab[core][rows].T
            sg[core][slots % 128, slots // 128] = score0_c[core][rows]

    r2 = _run(ex2, {"at2": at2, "bt": bt_all, "sg": sg})
    within = r2["idx"][:, :, ::8]                                 # [8, 128, NST]

    match01 = np.empty((B, N), dtype=np.int32)
    score0 = np.empty((B, N), dtype=np.float32)
    valid = np.empty((B, N), dtype=bool)
    colmax = r1["colp"].reshape(B, 2, M).max(axis=1)              # [B, M]

    for core in range(NCORES):
        b, h = divmod(core, 2)
        s = score0_c[core]
        sl = slot_of_row[core]
        m = cstar_c[core] * 1024 + \
            within[core][sl % 128, sl // 128].astype(np.int64)
        sel = slice(h * HALF, (h + 1) * HALF)
        score0[b, sel] = s
        match01[b, sel] = m.astype(np.int32)
        valid[b, sel] = (s > 0.1) & (s == colmax[b][m])

    for core, row in overflow:                                    # ~never taken
        b, h = divmod(core, 2)
        simrow = a_slab[core][row] @ desc1[b].T
        n = h * HALF + row
        match01[b, n] = int(simrow.argmax())
        score0[b, n] = simrow.max()
        valid[b, n] = (score0[b, n] > 0.1) & \
                      (score0[b, n] == colmax[b][match01[b, n]])

    return match01, score0, valid



# revision 5
# speedup vs baseline: 1.5680x; 1.5680x over previous
"""Trainium2 Bass kernel for DescriptorMatcher (mutual nearest neighbor matching).

Problem: given desc0 [B,N,D], desc1 [B,M,D] (B=4, N=M=8192, D=128, fp32):
    sim     = desc0 @ desc1^T                      [B,N,M]
    score0  = max_m sim                            [B,N]
    match01 = argmax_m sim                         [B,N]
    match10 = argmax_n sim                         [B,M]
    valid   = (match10[match01[n]] == n) & (score0 > 0.1)
returns (match01, score0, valid).

Key reformulation (same as the fp32 baseline): the mutual check never needs
match10 indices:
    match10[match01[n]] == n  <=>  score0[n] == colmax[match01[n]]
(max chains over the same on-device fp32 values are exact, so equality holds
exactly for mutual pairs and only for them).

Matmuls run in fp32r (full PE rate; ~1.6e-4 rel rounding). All downstream
max/argmax chains compare the SAME on-device fp32 PSUM values, so the
equality trick and phase-1/phase-2 consistency are bit-exact; only the
match01-vs-fp32-reference flips remain (~0.03% of rows, well under the 2e-2
gate).

Sharding: 8 cores = 4 batches x 2 row-halves (phase 1) and
4 batches x 2 column-halves (phase 2).

Phase 1 (per core): for each of 32 n-tiles [128 rows x 8192 cols]:
    PE:   16 fp32r matmuls -> PSUM [128,2048] x4
    ACT:  copy PSUM -> SBUF row buffer [128, 8192] (fp32)
    DVE:  one fused reduce [128,32,256]-view -> CM (32 chunk maxima/row)
    Pool: tensor_reduce(axis=C) row -> colpart[t] (per-tile column max)
  tail: Pool axis-C over colpart[0:32] -> exact per-column max (this half).
  Host: score0 = CM.max, c* = CM.argmax (first occurrence), colmax = max of
  the two halves' column maxes.

Phase 2 (per core, ~25 us): rows of batch b whose winning 256-wide chunk
lies in column-half h2, grouped by chunk; recompute sim[:, chunk] with
identically-laid-out fp32r matmuls (bit-exact per element) and
max_index(score, chunk) gives the exact first-occurrence within-chunk
position. match01 = chunk*256 + within.

Rows overflowing a group's padded capacity (needs >384 of ~256 expected rows
sharing one winning chunk; ~8 sigma) fall back to a host-side recompute.
"""

import numpy as np

import concourse.bass as bass  # noqa: F401  (bass must import before tile)
import concourse.mybir as mybir
import concourse.tile as tile
from concourse import bacc

B, N, M, D = 4, 8192, 8192, 128
NCORES = 8
HALF = N // 2          # rows per phase-1 core
NT = HALF // 128       # 32 n-tiles per core
CW = 256               # row-side chunk width (phase-2 recompute width)
NCHUNK = M // CW       # 32 chunks per row
CPT = NCHUNK           # chunks per tile row (same thing)
PAD = 384              # phase-2 rows per chunk-group (mean 256, sigma ~16)
GRP = 16               # chunk-groups per phase-2 core (NCHUNK/2)
NST = GRP * PAD // 128  # 48 phase-2 sub-tiles


def _build1():
    f32 = mybir.dt.float32
    f32r = mybir.dt.float32r
    nc = bacc.Bacc("TRN2", target_bir_lowering=False, debug=False,
                   num_devices=NCORES)
    at = nc.dram_tensor("at", [D, HALF], f32, kind="ExternalInput").ap()
    bt = nc.dram_tensor("bt", [D, M], f32, kind="ExternalInput").ap()
    cm_o = nc.dram_tensor("cm", [128, NT * CPT], f32, kind="ExternalOutput").ap()
    colp_o = nc.dram_tensor("colp", [NT, M], f32, kind="ExternalOutput").ap()

    with tile.TileContext(nc) as tc:
        with tc.tile_pool(name="big", bufs=1) as big, \
             tc.tile_pool(name="rows", bufs=3) as rows, \
             tc.tile_pool(name="cps", bufs=4) as cps, \
             tc.tile_pool(name="ps", bufs=2, space="PSUM") as ps:
            atb = big.tile([128, HALF], f32r, name="atb")
            btb = big.tile([128, M], f32r, name="btb")
            for c in range(0, HALF, 1024):
                nc.sync.dma_start(atb[:, c:c + 1024],
                                  at[:, c:c + 1024].bitcast(f32r))
            for c in range(0, M, 1024):
                nc.sync.dma_start(btb[:, c:c + 1024],
                                  bt[:, c:c + 1024].bitcast(f32r))
            cm_all = big.tile([128, NT * CPT], f32, name="cm_all")
            for t in range(NT):
                row = rows.tile([128, M], f32, tag="row", name="row")
                for c in range(4):
                    pt = ps.tile([128, 2048], f32, tag="pt", name="pt")
                    for j in range(4):
                        mlo = c * 2048 + j * 512
                        nc.tensor.matmul(pt[:, j * 512:(j + 1) * 512],
                                         atb[:, t * 128:(t + 1) * 128],
                                         btb[:, mlo:mlo + 512],
                                         start=True, stop=True)
                    nc.scalar.copy(row[:, c * 2048:(c + 1) * 2048], pt[:])
                v = row[:].rearrange("p (c w) -> p c w", w=CW)
                nc.vector.tensor_reduce(cm_all[:, t * CPT:(t + 1) * CPT], v,
                                        axis=mybir.AxisListType.X,
                                        op=mybir.AluOpType.max)
                for c in range(4):
                    cp = cps.tile([1, 2048], f32, tag="cp", name="cp")
                    nc.gpsimd.tensor_reduce(cp[:],
                                            row[:, c * 2048:(c + 1) * 2048],
                                            axis=mybir.AxisListType.C,
                                            op=mybir.AluOpType.max)
                    nc.sync.dma_start(
                        colp_o[t:t + 1, c * 2048:(c + 1) * 2048], cp[:])
            nc.sync.dma_start(cm_o[:], cm_all[:])
    nc.compile()
    return nc


def _build2():
    f32, f32r, u32 = mybir.dt.float32, mybir.dt.float32r, mybir.dt.uint32
    nc = bacc.Bacc("TRN2", target_bir_lowering=False, debug=False,
                   num_devices=NCORES)
    at2 = nc.dram_tensor("at2", [D, GRP * PAD], f32, kind="ExternalInput").ap()
    bt2 = nc.dram_tensor("bt2", [D, M // 2], f32, kind="ExternalInput").ap()
    sg = nc.dram_tensor("sg", [128, NST], f32, kind="ExternalInput").ap()
    idx_o = nc.dram_tensor("idx", [128, NST * 8], u32, kind="ExternalOutput").ap()
    with tile.TileContext(nc) as tc:
        with tc.tile_pool(name="big", bufs=1) as big, \
             tc.tile_pool(name="work", bufs=4) as work, \
             tc.tile_pool(name="ps", bufs=4, space="PSUM") as ps:
            a2b = big.tile([128, GRP * PAD], f32r, name="a2b")
            b2b = big.tile([128, M // 2], f32r, name="b2b")
            sgb = big.tile([128, NST], f32, name="sgb")
            nc.sync.dma_start(sgb[:], sg[:])
            # chunked so group 0's matmuls start before all input has landed
            for c in range(0, GRP * PAD, 1024):
                w = min(1024, GRP * PAD - c)
                nc.sync.dma_start(a2b[:, c:c + w], at2[:, c:c + w].bitcast(f32r))
            for c in range(0, M // 2, 1024):
                nc.sync.dma_start(b2b[:, c:c + 1024],
                                  bt2[:, c:c + 1024].bitcast(f32r))
            idx8 = big.tile([128, NST * 8], u32, name="idx8")
            KP = PAD // 128
            for g in range(GRP):
                for k in range(KP):
                    st = g * KP + k
                    pt = ps.tile([128, CW], f32, tag="pt", name="pt")
                    nc.tensor.matmul(pt[:],
                                     a2b[:, st * 128:(st + 1) * 128],
                                     b2b[:, g * CW:(g + 1) * CW],
                                     start=True, stop=True)
                    sc8 = work.tile([128, 8], f32, tag="sc8", name="sc8")
                    nc.vector.tensor_copy(
                        sc8[:], sgb[:, st:st + 1].to_broadcast((128, 8)))
                    nc.vector.max_index(idx8[:, st * 8:(st + 1) * 8],
                                        sc8[:], pt[:])
            nc.sync.dma_start(idx_o[:], idx8[:])
    nc.compile()
    return nc


_cached = None


def _make_exec(nc):
    import jax
    from jax.sharding import Mesh, PartitionSpec
    from jax.experimental.shard_map import shard_map
    from concourse import bass2jax
    from concourse.bass2jax import _bass_exec_p

    partition_name = nc.partition_id_tensor.name if nc.partition_id_tensor else None
    in_names, out_names, out_avals, out_shapes = [], [], [], []
    for alloc in nc.m.functions[0].allocations:
        if not isinstance(alloc, mybir.MemoryLocationSet):
            continue
        name = alloc.memorylocations[0].name
        if alloc.kind == "ExternalInput":
            if name != partition_name:
                in_names.append(name)
        elif alloc.kind == "ExternalOutput":
            shape = tuple(alloc.tensor_shape)
            dtype = mybir.dt.np(alloc.dtype)
            out_names.append(name)
            out_shapes.append((shape, dtype))
            out_avals.append(jax.core.ShapedArray(shape, dtype))
    n_params = len(in_names)
    n_outs = len(out_names)
    all_in_names = in_names + out_names
    if partition_name is not None:
        all_in_names = all_in_names + [partition_name]

    def _body(*args):
        operands = list(args)
        if partition_name is not None:
            operands.append(bass2jax.partition_id_tensor())
        outs = _bass_exec_p.bind(
            *operands, out_avals=tuple(out_avals), in_names=tuple(all_in_names),
            out_names=tuple(out_names), lowering_input_output_aliases=(),
            sim_require_finite=True, sim_require_nnan=True, nc=nc)
        return tuple(outs)

    devices = jax.devices()[:NCORES]
    mesh = Mesh(np.asarray(devices), ("core",))
    in_specs = (PartitionSpec("core"),) * (n_params + n_outs)
    out_specs = (PartitionSpec("core"),) * n_outs
    fn = jax.jit(shard_map(_body, mesh=mesh, in_specs=in_specs,
                           out_specs=out_specs, check_rep=False),
                 keep_unused=True)
    return {"fn": fn, "in_names": in_names, "out_names": out_names,
            "out_shapes": out_shapes, "nc": nc}


def _run(ex, ins):
    """ins: dict name -> [NCORES, *shape]; returns dict name -> [NCORES, *shape]."""
    concat_in = [np.ascontiguousarray(ins[n].reshape(-1, *ins[n].shape[2:]))
                 for n in ex["in_names"]]
    concat_zeros = [np.zeros((NCORES * s[0], *s[1:]), dt)
                    for (s, dt) in ex["out_shapes"]]
    out_arrs = ex["fn"](*concat_in, *concat_zeros)
    return {name: np.asarray(out_arrs[i]).reshape(NCORES, *ex["out_shapes"][i][0])
            for i, name in enumerate(ex["out_names"])}


def kernel(desc0, desc1):
    global _cached
    desc0 = np.asarray(desc0, dtype=np.float32)
    desc1 = np.asarray(desc1, dtype=np.float32)
    assert desc0.shape == (B, N, D) and desc1.shape == (B, M, D)

    if _cached is None:
        _cached = (_make_exec(_build1()), _make_exec(_build2()))
    ex1, ex2 = _cached

    a_slab = np.stack([desc0[b, h * HALF:(h + 1) * HALF]
                       for b in range(B) for h in range(2)])      # [8,4096,128]
    bt_all = np.stack([desc1[b].transpose(1, 0)
                       for b in range(B) for h in range(2)])      # [8,128,8192]
    at_all = a_slab.transpose(0, 2, 1)                            # [8,128,4096]

    r1 = _run(ex1, {"at": at_all, "bt": bt_all})

    # host glue: score/chunk-argmax + grouping for phase 2
    cm = r1["cm"].reshape(NCORES, 128, NT, CPT).transpose(0, 2, 1, 3) \
                 .reshape(NCORES, HALF, CPT)
    # full-batch views [B, N, CPT]
    cm_b = cm.reshape(B, 2 * HALF, CPT)
    score0 = cm_b.max(axis=2)                                     # [B, N]
    cstar = cm_b.argmax(axis=2)                                   # [B, N]
    colmax = r1["colp"].reshape(B, 2 * NT, M).max(axis=1)         # [B, M]

    at2 = np.zeros((NCORES, D, GRP * PAD), np.float32)
    sg = np.full((NCORES, 128, NST), 1e30, np.float32)
    slot_of_row = np.full((B, N), -1, np.int64)
    core_of_row = np.full((B, N), -1, np.int64)
    overflow = []                                                 # (b, n)
    for b in range(B):
        for g in range(NCHUNK):
            rws = np.nonzero(cstar[b] == g)[0]
            core = 2 * b + (g >= GRP)
            gl = g % GRP                                          # local group
            if len(rws) > PAD:
                overflow.extend((b, n) for n in rws[PAD:])
                rws = rws[:PAD]
            slots = gl * PAD + np.arange(len(rws))
            slot_of_row[b, rws] = slots
            core_of_row[b, rws] = core
            at2[core][:, slots] = desc0[b, rws].T
            sg[core][slots % 128, slots // 128] = score0[b, rws]

    bt2_all = np.stack([desc1[b].T[:, h2 * (M // 2):(h2 + 1) * (M // 2)]
                        for b in range(B) for h2 in range(2)])    # [8,128,4096]
    r2 = _run(ex2, {"at2": at2, "bt2": bt2_all, "sg": sg})
    within = r2["idx"][:, :, ::8]                                 # [8, 128, NST]

    sl = slot_of_row
    cr = core_of_row
    w = within[cr, sl % 128, sl // 128].astype(np.int64)          # [B, N]
    match01 = (cstar * CW + w).astype(np.int32)
    valid = (score0 > 0.1) & \
            (score0 == np.take_along_axis(colmax, match01.astype(np.int64),
                                          axis=1))

    for b, n in overflow:                                         # ~never taken
        simrow = desc0[b, n] @ desc1[b].T
        j = int(simrow.argmax())
        s = simrow.max()
        col = desc0[b] @ desc1[b, j]
        match01[b, n] = j
        score0[b, n] = s
        valid[b, n] = (s > 0.1) & (int(col.argmax()) == n)

    return match01, score0.astype(np.float32), valid


# revision 16
# speedup vs baseline: 2.0751x; 1.3234x over previous
"""Trainium2 Bass kernel for DescriptorMatcher (mutual nearest neighbor matching).

Problem: given desc0 [B,N,D], desc1 [B,M,D] (B=4, N=M=8192, D=128, fp32):
    sim     = desc0 @ desc1^T                      [B,N,M]
    score0  = max_m sim                            [B,N]
    match01 = argmax_m sim                         [B,N]
    match10 = argmax_n sim                         [B,M]
    valid   = (match10[match01[n]] == n) & (score0 > 0.1)
returns (match01, score0, valid).

Key reformulation: the mutual check never needs match10 indices:
    match10[match01[n]] == n  <=>  score0[n] == colmax[match01[n]]
(max chains over the same on-device fp32 values are exact).

Matmuls run in fp32r (full PE rate; ~1.6e-4 rel rounding). All downstream
max/argmax chains compare the SAME on-device fp32 PSUM values, so the
equality trick and phase-1/phase-2 consistency hold bit-exactly; only
match01-vs-fp32-reference flips remain (~0.03% of rows, well under the
2e-2 gate).

Sharding: 8 cores = 4 batches x 2 row-halves (phase 1), then
4 batches x 2 column-halves (phase 2).

Phase 1 (per core), per 128-row tile [128 x 8192]:
    PE:   16 fp32r matmuls -> PSUM [128,2048] x4
    ACT:  copy PSUM -> SBUF row buffer (fp32)
    DVE:  16 tensor_scalar(identity, accum_out=max) ops -> CM chunk maxima
          (2x_2p mode: 0.5 cyc/elem) + colacc = max(colacc, row[:, :WD])
    Pool: tensor_reduce(axis=C) on row[:, WD:] -> per-tile column partials
          (software partition reduce, ~1.44 ns/col), DMA'd per tile
  tail: one axis-C reduce of colacc -> exact column max for cols [0, WD).
  Host: score0 = CM.max, c* = CM.argmax (first occurrence), colmax from
  colacc-final + per-tile partials.

Phase 2 (per core): rows of batch b whose winning 512-wide chunk lies in
column-half h2, grouped by chunk; recompute sim[:, chunk] with
identically-laid-out fp32r matmuls (bit-exact per element), then
max_index(score, chunk) gives the exact first-occurrence within-chunk
position. match01 = chunk*512 + within.

Rows overflowing a group's padded capacity (needs >640 of ~512 expected
rows sharing one winning chunk; ~6 sigma) fall back to a host recompute.
"""

import numpy as np

import concourse.bass as bass  # noqa: F401  (bass must import before tile)
import concourse.mybir as mybir
import concourse.tile as tile
from concourse import bacc, bass_isa

B, N, M, D = 4, 8192, 8192, 128
NCORES = 8
HALF = N // 2          # rows per phase-1 core
NT = HALF // 128       # 32 n-tiles per core
CW = 512               # row-side chunk width (phase-2 recompute width)
NCHUNK = M // CW       # 16 chunks per row
WD = 2400              # colacc columns on DVE; Pool handles [WD, M)
PAD = 640              # phase-2 rows per chunk-group (mean 512, sigma ~22)
GRP = NCHUNK // 2      # 8 chunk-groups per phase-2 core
NST = GRP * PAD // 128  # 40 phase-2 sub-tiles


def _build1():
    f32 = mybir.dt.float32
    f32r = mybir.dt.float32r
    nc = bacc.Bacc("TRN2", target_bir_lowering=False, debug=False,
                   num_devices=NCORES)
    at = nc.dram_tensor("at", [D, HALF], f32, kind="ExternalInput").ap()
    bt = nc.dram_tensor("bt", [D, M], f32, kind="ExternalInput").ap()
    cm_o = nc.dram_tensor("cm", [128, NT * NCHUNK], f32,
                          kind="ExternalOutput").ap()
    # rows 0..NT-1: per-tile Pool column partials (cols [WD, M) valid);
    # row NT: colacc final (cols [0, WD) valid)
    colp_o = nc.dram_tensor("colp", [NT + 1, M], f32,
                            kind="ExternalOutput").ap()

    with tile.TileContext(nc) as tc:
        with tc.tile_pool(name="big", bufs=1) as big, \
             tc.tile_pool(name="rows", bufs=2) as rows, \
             tc.tile_pool(name="cps", bufs=2) as cps, \
             tc.tile_pool(name="dmy", bufs=2) as dmy, \
             tc.tile_pool(name="ps", bufs=2, space="PSUM") as ps:
            atb = big.tile([128, HALF], f32r, name="atb")
            btb = big.tile([128, M], f32r, name="btb")
            # tile 0 needs at[:, 0:128] and then bt chunks in matmul order;
            # front-load those so the PE starts as early as possible
            nc.sync.dma_start(atb[:, 0:1024], at[:, 0:1024].bitcast(f32r))
            for c in range(0, M, 1024):
                nc.sync.dma_start(btb[:, c:c + 1024],
                                  bt[:, c:c + 1024].bitcast(f32r))
            for c in range(1024, HALF, 1024):
                nc.sync.dma_start(atb[:, c:c + 1024],
                                  at[:, c:c + 1024].bitcast(f32r))
            cm_all = big.tile([128, NT * NCHUNK], f32, name="cm_all")
            colacc = big.tile([128, WD], f32, name="colacc")
            for t in range(NT):
                row = rows.tile([128, M], f32, tag="row", name="row")
                for c in range(4):
                    pt = ps.tile([128, 2048], f32, tag="pt", name="pt")
                    for j in range(4):
                        mlo = c * 2048 + j * 512
                        nc.tensor.matmul(pt[:, j * 512:(j + 1) * 512],
                                         atb[:, t * 128:(t + 1) * 128],
                                         btb[:, mlo:mlo + 512],
                                         start=True, stop=True)
                    nc.scalar.copy(row[:, c * 2048:(c + 1) * 2048], pt[:])
                # row side: chunk maxima via identity tensor_scalar with
                # max-accumulator (2x_2p: all-SBUF operands)
                for c in range(NCHUNK):
                    dummy = dmy.tile([128, CW], f32, tag="dmy", name="dmy")
                    nc.vector.tensor_scalar(
                        dummy[:], row[:, c * CW:(c + 1) * CW], 1.0, None,
                        op0=mybir.AluOpType.mult, op1=mybir.AluOpType.max,
                        accum_out=cm_all[:, t * NCHUNK + c:t * NCHUNK + c + 1])
                # column side, DVE part
                if t == 0:
                    nc.vector.tensor_copy(colacc[:], row[:, 0:WD])
                else:
                    nc.vector.tensor_tensor(colacc[:], colacc[:],
                                            row[:, 0:WD],
                                            op=mybir.AluOpType.max)
                # column side, Pool part: software partition reduce
                cp = cps.tile([128, M - WD], f32, tag="cp", name="cp")
                nc.gpsimd.partition_all_reduce(cp[:], row[:, WD:M],
                                               channels=128,
                                               reduce_op=bass_isa.ReduceOp.max)
                nc.sync.dma_start(colp_o[t:t + 1, WD:M], cp[0:1, :])
            cfin = cps.tile([128, WD], f32, tag="cfin", name="cfin")
            nc.gpsimd.partition_all_reduce(cfin[:], colacc[:], channels=128,
                                           reduce_op=bass_isa.ReduceOp.max)
            nc.sync.dma_start(colp_o[NT:NT + 1, 0:WD], cfin[0:1, :])
            nc.sync.dma_start(cm_o[:], cm_all[:])
    nc.compile()
    return nc


def _build2():
    f32, f32r, u32 = mybir.dt.float32, mybir.dt.float32r, mybir.dt.uint32
    nc = bacc.Bacc("TRN2", target_bir_lowering=False, debug=False,
                   num_devices=NCORES)
    at2 = nc.dram_tensor("at2", [D, GRP * PAD], f32, kind="ExternalInput").ap()
    bt2 = nc.dram_tensor("bt2", [D, M // 2], f32, kind="ExternalInput").ap()
    sg = nc.dram_tensor("sg", [128, NST * 8], f32, kind="ExternalInput").ap()
    idx_o = nc.dram_tensor("idx", [128, NST * 8], u32, kind="ExternalOutput").ap()
    with tile.TileContext(nc) as tc:
        with tc.tile_pool(name="big", bufs=1) as big, \
             tc.tile_pool(name="ps", bufs=4, space="PSUM") as ps:
            a2b = big.tile([128, GRP * PAD], f32r, name="a2b")
            b2b = big.tile([128, M // 2], f32r, name="b2b")
            sgb = big.tile([128, NST * 8], f32, name="sgb")
            nc.sync.dma_start(sgb[:], sg[:])
            # interleave so group 0's matmuls start before all input lands
            na = (GRP * PAD + 1023) // 1024
            nb = (M // 2) // 1024
            for i in range(max(na, nb)):
                if i < na:
                    c = i * 1024
                    w = min(1024, GRP * PAD - c)
                    nc.sync.dma_start(a2b[:, c:c + w],
                                      at2[:, c:c + w].bitcast(f32r))
                if i < nb:
                    c = i * 1024
                    nc.sync.dma_start(b2b[:, c:c + 1024],
                                      bt2[:, c:c + 1024].bitcast(f32r))
            idx8 = big.tile([128, NST * 8], u32, name="idx8")
            KP = PAD // 128
            for g in range(GRP):
                for k in range(KP):
                    st = g * KP + k
                    pt = ps.tile([128, CW], f32, tag="pt", name="pt")
                    nc.tensor.matmul(pt[:],
                                     a2b[:, st * 128:(st + 1) * 128],
                                     b2b[:, g * CW:(g + 1) * CW],
                                     start=True, stop=True)
                    nc.vector.max_index(idx8[:, st * 8:(st + 1) * 8],
                                        sgb[:, st * 8:(st + 1) * 8], pt[:])
            nc.sync.dma_start(idx_o[:], idx8[:])
    nc.compile()
    return nc


_cached = None


def _make_exec(nc):
    import jax
    from jax.sharding import Mesh, PartitionSpec
    from jax.experimental.shard_map import shard_map
    from concourse import bass2jax
    from concourse.bass2jax import _bass_exec_p

    partition_name = nc.partition_id_tensor.name if nc.partition_id_tensor else None
    in_names, out_names, out_avals, out_shapes = [], [], [], []
    for alloc in nc.m.functions[0].allocations:
        if not isinstance(alloc, mybir.MemoryLocationSet):
            continue
        name = alloc.memorylocations[0].name
        if alloc.kind == "ExternalInput":
            if name != partition_name:
                in_names.append(name)
        elif alloc.kind == "ExternalOutput":
            shape = tuple(alloc.tensor_shape)
            dtype = mybir.dt.np(alloc.dtype)
            out_names.append(name)
            out_shapes.append((shape, dtype))
            out_avals.append(jax.core.ShapedArray(shape, dtype))
    n_params = len(in_names)
    n_outs = len(out_names)
    all_in_names = in_names + out_names
    if partition_name is not None:
        all_in_names = all_in_names + [partition_name]

    def _body(*args):
        operands = list(args)
        if partition_name is not None:
            operands.append(bass2jax.partition_id_tensor())
        outs = _bass_exec_p.bind(
            *operands, out_avals=tuple(out_avals), in_names=tuple(all_in_names),
            out_names=tuple(out_names), lowering_input_output_aliases=(),
            sim_require_finite=True, sim_require_nnan=True, nc=nc)
        return tuple(outs)

    devices = jax.devices()[:NCORES]
    mesh = Mesh(np.asarray(devices), ("core",))
    in_specs = (PartitionSpec("core"),) * (n_params + n_outs)
    out_specs = (PartitionSpec("core"),) * n_outs
    fn = jax.jit(shard_map(_body, mesh=mesh, in_specs=in_specs,
                           out_specs=out_specs, check_rep=False),
                 keep_unused=True)
    return {"fn": fn, "in_names": in_names, "out_names": out_names,
            "out_shapes": out_shapes, "nc": nc}


def _run(ex, ins):
    """ins: dict name -> [NCORES, *shape]; returns dict name -> [NCORES, *shape]."""
    concat_in = [np.ascontiguousarray(ins[n].reshape(-1, *ins[n].shape[2:]))
                 for n in ex["in_names"]]
    concat_zeros = [np.zeros((NCORES * s[0], *s[1:]), dt)
                    for (s, dt) in ex["out_shapes"]]
    out_arrs = ex["fn"](*concat_in, *concat_zeros)
    return {name: np.asarray(out_arrs[i]).reshape(NCORES, *ex["out_shapes"][i][0])
            for i, name in enumerate(ex["out_names"])}


def kernel(desc0, desc1):
    global _cached
    desc0 = np.asarray(desc0, dtype=np.float32)
    desc1 = np.asarray(desc1, dtype=np.float32)
    assert desc0.shape == (B, N, D) and desc1.shape == (B, M, D)

    if _cached is None:
        _cached = (_make_exec(_build1()), _make_exec(_build2()))
    ex1, ex2 = _cached

    a_slab = np.stack([desc0[b, h * HALF:(h + 1) * HALF]
                       for b in range(B) for h in range(2)])      # [8,4096,128]
    bt_all = np.stack([desc1[b].transpose(1, 0)
                       for b in range(B) for h in range(2)])      # [8,128,8192]
    at_all = a_slab.transpose(0, 2, 1)                            # [8,128,4096]

    r1 = _run(ex1, {"at": at_all, "bt": bt_all})

    # host glue: score/chunk-argmax + grouping for phase 2
    cm = r1["cm"].reshape(NCORES, 128, NT, NCHUNK).transpose(0, 2, 1, 3) \
                 .reshape(NCORES, HALF, NCHUNK)
    cm_b = cm.reshape(B, N, NCHUNK)
    score0 = cm_b.max(axis=2)                                     # [B, N]
    cstar = cm_b.argmax(axis=2)                                   # [B, N]
    colp = r1["colp"].reshape(B, 2, NT + 1, M)
    colmax = np.empty((B, M), np.float32)
    colmax[:, :WD] = colp[:, :, NT, :WD].max(axis=1)
    colmax[:, WD:] = colp[:, :, :NT, WD:].max(axis=(1, 2))

    at2 = np.zeros((NCORES, D, GRP * PAD), np.float32)
    sg = np.full((NCORES, 128, NST), 1e30, np.float32)
    slot_of_row = np.full((B, N), -1, np.int64)
    core_of_row = np.full((B, N), 0, np.int64)
    overflow = []                                                 # (b, n)
    for b in range(B):
        for g in range(NCHUNK):
            rws = np.nonzero(cstar[b] == g)[0]
            core = 2 * b + (g >= GRP)
            gl = g % GRP                                          # local group
            if len(rws) > PAD:
                overflow.extend((b, n) for n in rws[PAD:])
                rws = rws[:PAD]
            slots = gl * PAD + np.arange(len(rws))
            slot_of_row[b, rws] = slots
            core_of_row[b, rws] = core
            at2[core][:, slots] = desc0[b, rws].T
            sg[core][slots % 128, slots // 128] = score0[b, rws]

    bt2_all = np.stack([desc1[b].T[:, h2 * (M // 2):(h2 + 1) * (M // 2)]
                        for b in range(B) for h2 in range(2)])    # [8,128,4096]
    r2 = _run(ex2, {"at2": at2, "bt2": bt2_all, "sg": sg})
    within = r2["idx"][:, :, ::8]                                 # [8, 128, NST]

    sl = np.maximum(slot_of_row, 0)
    cr = core_of_row
    w = within[cr, sl % 128, sl // 128].astype(np.int64)          # [B, N]
    match01 = (cstar * CW + w).astype(np.int32)
    valid = (score0 > 0.1) & \
            (score0 == np.take_along_axis(colmax, match01.astype(np.int64),
                                          axis=1))

    for b, n in overflow:                                         # ~never taken
        simrow = desc0[b, n] @ desc1[b].T
        j = int(simrow.argmax())
        s = simrow.max()
        col = desc0[b] @ desc1[b, j]
        match01[b, n] = j
        score0[b, n] = np.float32(s)
        valid[b, n] = (s > 0.1) & (int(col.argmax()) == n)

    return match01, score0.astype(np.float32), valid


# revision 29
# speedup vs baseline: 2.1074x; 1.0156x over previous
"""Trainium2 Bass kernel for DescriptorMatcher (mutual nearest neighbor matching).

Problem: given desc0 [B,N,D], desc1 [B,M,D] (B=4, N=M=8192, D=128, fp32):
    sim     = desc0 @ desc1^T                      [B,N,M]
    score0  = max_m sim                            [B,N]
    match01 = argmax_m sim                         [B,N]
    match10 = argmax_n sim                         [B,M]
    valid   = (match10[match01[n]] == n) & (score0 > 0.1)
returns (match01, score0, valid).

Key reformulation: the mutual check never needs match10 indices:
    match10[match01[n]] == n  <=>  score0[n] == colmax[match01[n]]
(max chains over the same on-device fp32 values are exact).

Matmuls run in fp32r (full PE rate; ~1.6e-4 rel rounding). All downstream
max/argmax chains compare the SAME on-device fp32 PSUM values, so the
equality trick and phase-1/phase-2 consistency hold bit-exactly; only
match01-vs-fp32-reference flips remain (~0.03% of rows, well under the
2e-2 gate).

Sharding: 8 cores = 4 batches x 2 row-halves (phase 1), then
4 batches x 2 column-halves (phase 2).

Phase 1 (per core), per 128-row tile [128 x 8192]:
    PE:   16 fp32r matmuls -> PSUM [128,2048] x4
    ACT:  copy PSUM -> SBUF row buffer (fp32)
    DVE:  16 tensor_scalar(identity, accum_out=max) ops -> CM chunk maxima
          (2x_2p mode: 0.5 cyc/elem) + colacc = max(colacc, row[:, :WD])
    Pool: tensor_reduce(axis=C) on row[:, WD:] -> per-tile column partials
          (software partition reduce, ~1.44 ns/col), DMA'd per tile
  tail: one axis-C reduce of colacc -> exact column max for cols [0, WD).
  Host: score0 = CM.max, c* = CM.argmax (first occurrence), colmax from
  colacc-final + per-tile partials.

Phase 2 (per core): rows of batch b whose winning 512-wide chunk lies in
column-half h2, grouped by chunk; recompute sim[:, chunk] with
identically-laid-out fp32r matmuls (bit-exact per element), then
max_index(score, chunk) gives the exact first-occurrence within-chunk
position. match01 = chunk*512 + within.

Rows overflowing a group's padded capacity (needs >640 of ~512 expected
rows sharing one winning chunk; ~6 sigma) fall back to a host recompute.
"""

import numpy as np

import concourse.bass as bass  # noqa: F401  (bass must import before tile)
import concourse.mybir as mybir
import concourse.tile as tile
from concourse import bacc, bass_isa

B, N, M, D = 4, 8192, 8192, 128
NCORES = 8
HALF = N // 2          # rows per phase-1 core
NT = HALF // 128       # 32 n-tiles per core
CW = 512               # row-side chunk width (phase-2 recompute width)
NCHUNK = M // CW       # 16 chunks per row
WD = 2624              # colacc columns on DVE; Pool handles [WD, M)
PAD = 640              # phase-2 rows per chunk-group (mean 512, sigma ~22)
GRP = NCHUNK // 2      # 8 chunk-groups per phase-2 core
NST = GRP * PAD // 128  # 40 phase-2 sub-tiles


def _build1():
    f32 = mybir.dt.float32
    f32r = mybir.dt.float32r
    nc = bacc.Bacc("TRN2", target_bir_lowering=False, debug=False,
                   num_devices=NCORES)
    at = nc.dram_tensor("at", [D, HALF], f32, kind="ExternalInput").ap()
    bt = nc.dram_tensor("bt", [D, M], f32, kind="ExternalInput").ap()
    cm_o = nc.dram_tensor("cm", [128, NT * NCHUNK], f32,
                          kind="ExternalOutput").ap()
    # rows 0..NT-1: per-tile Pool column partials (cols [WD, M) valid);
    # row NT: colacc final (cols [0, WD) valid)
    colp_o = nc.dram_tensor("colp", [NT + 1, M], f32,
                            kind="ExternalOutput").ap()

    with tile.TileContext(nc) as tc:
        with tc.tile_pool(name="big", bufs=1) as big, \
             tc.tile_pool(name="rows", bufs=3) as rows, \
             tc.tile_pool(name="dmy", bufs=2) as dmy, \
             tc.tile_pool(name="ps", bufs=2, space="PSUM") as ps:
            atb = big.tile([128, HALF], f32r, name="atb")
            btb = big.tile([128, M], f32r, name="btb")
            # tile 0 needs at[:, 0:128] and then bt chunks in matmul order;
            # front-load tiny slices of those so the PE starts ASAP
            nc.sync.dma_start(atb[:, 0:128], at[:, 0:128].bitcast(f32r))
            nc.sync.dma_start(btb[:, 0:512], bt[:, 0:512].bitcast(f32r))
            nc.sync.dma_start(btb[:, 512:1024], bt[:, 512:1024].bitcast(f32r))
            for c in range(1024, M, 1024):
                nc.sync.dma_start(btb[:, c:c + 1024],
                                  bt[:, c:c + 1024].bitcast(f32r))
            nc.sync.dma_start(atb[:, 128:1024], at[:, 128:1024].bitcast(f32r))
            for c in range(1024, HALF, 1024):
                nc.sync.dma_start(atb[:, c:c + 1024],
                                  at[:, c:c + 1024].bitcast(f32r))
            cm_all = big.tile([128, NT * NCHUNK], f32, name="cm_all")
            colacc = big.tile([128, WD], f32, name="colacc")
            for t in range(NT):
                row = rows.tile([128, M], f32, tag="row", name="row")
                for c in range(4):
                    pt = ps.tile([128, 2048], f32, tag="pt", name="pt")
                    for j in range(4):
                        mlo = c * 2048 + j * 512
                        nc.tensor.matmul(pt[:, j * 512:(j + 1) * 512],
                                         atb[:, t * 128:(t + 1) * 128],
                                         btb[:, mlo:mlo + 512],
                                         start=True, stop=True)
                    nc.scalar.copy(row[:, c * 2048:(c + 1) * 2048], pt[:])
                # row side: chunk maxima via identity tensor_scalar with
                # max-accumulator (2x_2p: all-SBUF operands)
                for c in range(NCHUNK):
                    dummy = dmy.tile([128, CW], f32, tag="dmy", name="dmy")
                    nc.vector.tensor_scalar(
                        dummy[:], row[:, c * CW:(c + 1) * CW], 1.0, None,
                        op0=mybir.AluOpType.mult, op1=mybir.AluOpType.max,
                        accum_out=cm_all[:, t * NCHUNK + c:t * NCHUNK + c + 1])
                # column side, DVE part (skipped for the last tile: its
                # whole row goes through the Pool partial so the colacc
                # finalization can overlap tile NT-1)
                if t == 0:
                    nc.vector.tensor_copy(colacc[:], row[:, 0:WD])
                elif t < NT - 1:
                    nc.vector.tensor_tensor(colacc[:], colacc[:],
                                            row[:, 0:WD],
                                            op=mybir.AluOpType.max)
                # column side, Pool part: software partition reduce,
                # in place on the row buffer (nothing reads it afterwards).
                # Last tile: per-2048-chunk so each partial starts right
                # after its ACT copy, shortening the drain.
                if t < NT - 1:
                    nc.gpsimd.partition_all_reduce(
                        row[:, WD:M], row[:, WD:M], channels=128,
                        reduce_op=bass_isa.ReduceOp.max)
                    nc.sync.dma_start(colp_o[t:t + 1, WD:M], row[0:1, WD:M])
                else:
                    for c in range(4):
                        lo, hi = c * 2048, (c + 1) * 2048
                        nc.gpsimd.partition_all_reduce(
                            row[:, lo:hi], row[:, lo:hi], channels=128,
                            reduce_op=bass_isa.ReduceOp.max)
                        nc.sync.dma_start(colp_o[t:t + 1, lo:hi],
                                          row[0:1, lo:hi])
            nc.gpsimd.partition_all_reduce(colacc[:], colacc[:], channels=128,
                                           reduce_op=bass_isa.ReduceOp.max)
            nc.sync.dma_start(colp_o[NT:NT + 1, 0:WD], colacc[0:1, :])
            nc.sync.dma_start(cm_o[:, 0:(NT - 1) * NCHUNK],
                              cm_all[:, 0:(NT - 1) * NCHUNK])
            nc.sync.dma_start(cm_o[:, (NT - 1) * NCHUNK:],
                              cm_all[:, (NT - 1) * NCHUNK:])
    nc.compile()
    return nc


def _build2():
    f32, f32r, u32 = mybir.dt.float32, mybir.dt.float32r, mybir.dt.uint32
    nc = bacc.Bacc("TRN2", target_bir_lowering=False, debug=False,
                   num_devices=NCORES)
    at2 = nc.dram_tensor("at2", [D, GRP * PAD], f32, kind="ExternalInput").ap()
    bt2 = nc.dram_tensor("bt2", [D, M // 2], f32, kind="ExternalInput").ap()
    sg = nc.dram_tensor("sg", [128, NST * 8], f32, kind="ExternalInput").ap()
    idx_o = nc.dram_tensor("idx", [128, NST * 8], u32, kind="ExternalOutput").ap()
    with tile.TileContext(nc) as tc:
        with tc.tile_pool(name="big", bufs=1) as big, \
             tc.tile_pool(name="ps", bufs=4, space="PSUM") as ps:
            a2b = big.tile([128, GRP * PAD], f32r, name="a2b")
            b2b = big.tile([128, M // 2], f32r, name="b2b")
            sgb = big.tile([128, NST * 8], f32, name="sgb")
            nc.sync.dma_start(a2b[:, 0:128], at2[:, 0:128].bitcast(f32r))
            nc.sync.dma_start(b2b[:, 0:512], bt2[:, 0:512].bitcast(f32r))
            nc.sync.dma_start(sgb[:], sg[:])
            # interleave so group 0's matmuls start before all input lands
            na = (GRP * PAD + 1023) // 1024
            nb = (M // 2) // 1024
            for i in range(max(na, nb)):
                if i < na:
                    c = i * 1024
                    lo = 128 if i == 0 else 0
                    w = min(1024, GRP * PAD - c)
                    nc.sync.dma_start(a2b[:, c + lo:c + w],
                                      at2[:, c + lo:c + w].bitcast(f32r))
                if i < nb:
                    c = i * 1024
                    lo = 512 if i == 0 else 0
                    nc.sync.dma_start(b2b[:, c + lo:c + 1024],
                                      bt2[:, c + lo:c + 1024].bitcast(f32r))
            idx8 = big.tile([128, NST * 8], u32, name="idx8")
            KP = PAD // 128
            for g in range(GRP):
                for k in range(KP):
                    st = g * KP + k
                    pt = ps.tile([128, CW], f32, tag="pt", name="pt")
                    nc.tensor.matmul(pt[:],
                                     a2b[:, st * 128:(st + 1) * 128],
                                     b2b[:, g * CW:(g + 1) * CW],
                                     start=True, stop=True)
                    nc.vector.max_index(idx8[:, st * 8:(st + 1) * 8],
                                        sgb[:, st * 8:(st + 1) * 8], pt[:])
            nc.sync.dma_start(idx_o[:], idx8[:])
    nc.compile()
    return nc


_cached = None


def _make_exec(nc):
    import jax
    from jax.sharding import Mesh, PartitionSpec
    from jax.experimental.shard_map import shard_map
    from concourse import bass2jax
    from concourse.bass2jax import _bass_exec_p

    partition_name = nc.partition_id_tensor.name if nc.partition_id_tensor else None
    in_names, out_names, out_avals, out_shapes = [], [], [], []
    for alloc in nc.m.functions[0].allocations:
        if not isinstance(alloc, mybir.MemoryLocationSet):
            continue
        name = alloc.memorylocations[0].name
        if alloc.kind == "ExternalInput":
            if name != partition_name:
                in_names.append(name)
        elif alloc.kind == "ExternalOutput":
            shape = tuple(alloc.tensor_shape)
            dtype = mybir.dt.np(alloc.dtype)
            out_names.append(name)
            out_shapes.append((shape, dtype))
            out_avals.append(jax.core.ShapedArray(shape, dtype))
    n_params = len(in_names)
    n_outs = len(out_names)
    all_in_names = in_names + out_names
    if partition_name is not None:
        all_in_names = all_in_names + [partition_name]

    def _body(*args):
        operands = list(args)
        if partition_name is not None:
            operands.append(bass2jax.partition_id_tensor())
        outs = _bass_exec_p.bind(
            *operands, out_avals=tuple(out_avals), in_names=tuple(all_in_names),
            out_names=tuple(out_names), lowering_input_output_aliases=(),
            sim_require_finite=True, sim_require_nnan=True, nc=nc)
        return tuple(outs)

    devices = jax.devices()[:NCORES]
    mesh = Mesh(np.asarray(devices), ("core",))
    in_specs = (PartitionSpec("core"),) * (n_params + n_outs)
    out_specs = (PartitionSpec("core"),) * n_outs
    fn = jax.jit(shard_map(_body, mesh=mesh, in_specs=in_specs,
                           out_specs=out_specs, check_rep=False),
                 keep_unused=True)
    return {"fn": fn, "in_names": in_names, "out_names": out_names,
            "out_shapes": out_shapes, "nc": nc}


def _run(ex, ins):
    """ins: dict name -> [NCORES, *shape]; returns dict name -> [NCORES, *shape]."""
    concat_in = [np.ascontiguousarray(ins[n].reshape(-1, *ins[n].shape[2:]))
                 for n in ex["in_names"]]
    concat_zeros = [np.zeros((NCORES * s[0], *s[1:]), dt)
                    for (s, dt) in ex["out_shapes"]]
    out_arrs = ex["fn"](*concat_in, *concat_zeros)
    return {name: np.asarray(out_arrs[i]).reshape(NCORES, *ex["out_shapes"][i][0])
            for i, name in enumerate(ex["out_names"])}


def kernel(desc0, desc1):
    global _cached
    desc0 = np.asarray(desc0, dtype=np.float32)
    desc1 = np.asarray(desc1, dtype=np.float32)
    assert desc0.shape == (B, N, D) and desc1.shape == (B, M, D)

    if _cached is None:
        _cached = (_make_exec(_build1()), _make_exec(_build2()))
    ex1, ex2 = _cached

    a_slab = np.stack([desc0[b, h * HALF:(h + 1) * HALF]
                       for b in range(B) for h in range(2)])      # [8,4096,128]
    bt_all = np.stack([desc1[b].transpose(1, 0)
                       for b in range(B) for h in range(2)])      # [8,128,8192]
    at_all = a_slab.transpose(0, 2, 1)                            # [8,128,4096]

    r1 = _run(ex1, {"at": at_all, "bt": bt_all})

    # host glue: score/chunk-argmax + grouping for phase 2
    cm = r1["cm"].reshape(NCORES, 128, NT, NCHUNK).transpose(0, 2, 1, 3) \
                 .reshape(NCORES, HALF, NCHUNK)
    cm_b = cm.reshape(B, N, NCHUNK)
    score0 = cm_b.max(axis=2)                                     # [B, N]
    cstar = cm_b.argmax(axis=2)                                   # [B, N]
    colp = r1["colp"].reshape(B, 2, NT + 1, M)
    colmax = np.empty((B, M), np.float32)
    colmax[:, :WD] = np.maximum(colp[:, :, NT, :WD],
                                colp[:, :, NT - 1, :WD]).max(axis=1)
    colmax[:, WD:] = colp[:, :, :NT, WD:].max(axis=(1, 2))

    at2 = np.zeros((NCORES, D, GRP * PAD), np.float32)
    sg = np.full((NCORES, 128, NST), 1e30, np.float32)
    slot_of_row = np.full((B, N), -1, np.int64)
    core_of_row = np.full((B, N), 0, np.int64)
    overflow = []                                                 # (b, n)
    for b in range(B):
        for g in range(NCHUNK):
            rws = np.nonzero(cstar[b] == g)[0]
            core = 2 * b + (g >= GRP)
            gl = g % GRP                                          # local group
            if len(rws) > PAD:
                overflow.extend((b, n) for n in rws[PAD:])
                rws = rws[:PAD]
            slots = gl * PAD + np.arange(len(rws))
            slot_of_row[b, rws] = slots
            core_of_row[b, rws] = core
            at2[core][:, slots] = desc0[b, rws].T
            sg[core][slots % 128, slots // 128] = score0[b, rws]

    bt2_all = np.stack([desc1[b].T[:, h2 * (M // 2):(h2 + 1) * (M // 2)]
                        for b in range(B) for h2 in range(2)])    # [8,128,4096]
    sg8 = np.repeat(sg, 8, axis=2)                         # [8,128,NST*8]
    r2 = _run(ex2, {"at2": at2, "bt2": bt2_all, "sg": sg8})
    within = r2["idx"][:, :, ::8]                                 # [8, 128, NST]

    sl = np.maximum(slot_of_row, 0)
    cr = core_of_row
    w = within[cr, sl % 128, sl // 128].astype(np.int64)          # [B, N]
    match01 = (cstar * CW + w).astype(np.int32)
    valid = (score0 > 0.1) & \
            (score0 == np.take_along_axis(colmax, match01.astype(np.int64),
                                          axis=1))

    for b, n in overflow:                                         # ~never taken
        simrow = desc0[b, n] @ desc1[b].T
        j = int(simrow.argmax())
        s = simrow.max()
        col = desc0[b] @ desc1[b, j]
        match01[b, n] = j
        score0[b, n] = np.float32(s)
        valid[b, n] = (s > 0.1) & (int(col.argmax()) == n)

    return match01, score0.astype(np.float32), valid


# revision 35
# speedup vs baseline: 2.1216x; 1.0068x over previous
"""Trainium2 Bass kernel for DescriptorMatcher (mutual nearest neighbor matching).

Problem: given desc0 [B,N,D], desc1 [B,M,D] (B=4, N=M=8192, D=128, fp32):
    sim     = desc0 @ desc1^T                      [B,N,M]
    score0  = max_m sim                            [B,N]
    match01 = argmax_m sim                         [B,N]
    match10 = argmax_n sim                         [B,M]
    valid   = (match10[match01[n]] == n) & (score0 > 0.1)
returns (match01, score0, valid).

Key reformulation: the mutual check never needs match10 indices:
    match10[match01[n]] == n  <=>  score0[n] == colmax[match01[n]]
(max chains over the same on-device fp32 values are exact).

Matmuls run in fp32r (full PE rate; ~1.6e-4 rel rounding). All downstream
max/argmax chains compare the SAME on-device fp32 PSUM values, so the
equality trick and phase-1/phase-2 consistency hold bit-exactly; only
match01-vs-fp32-reference flips remain (~0.03% of rows, well under the
2e-2 gate).

Sharding: 8 cores = 4 batches x 2 row-halves (phase 1), then
4 batches x 2 column-halves (phase 2).

Phase 1 (per core), per 128-row tile [128 x 8192]:
    PE:   16 fp32r matmuls -> PSUM [128,2048] x4
    ACT:  copy PSUM -> SBUF row buffer (fp32)
    DVE:  16 tensor_scalar(identity, accum_out=max) ops -> CM chunk maxima
          (2x_2p mode: 0.5 cyc/elem) + colacc = max(colacc, row[:, :WD])
    Pool: tensor_reduce(axis=C) on row[:, WD:] -> per-tile column partials
          (software partition reduce, ~1.44 ns/col), DMA'd per tile
  tail: one axis-C reduce of colacc -> exact column max for cols [0, WD).
  Host: score0 = CM.max, c* = CM.argmax (first occurrence), colmax from
  colacc-final + per-tile partials.

Phase 2 (per core): rows of batch b whose winning 512-wide chunk lies in
column-half h2, grouped by chunk; recompute sim[:, chunk] with
identically-laid-out fp32r matmuls (bit-exact per element), then
max_index(score, chunk) gives the exact first-occurrence within-chunk
position. match01 = chunk*512 + within.

Rows overflowing a group's padded capacity (needs >640 of ~512 expected
rows sharing one winning chunk; ~6 sigma) fall back to a host recompute.
"""

import numpy as np

import concourse.bass as bass  # noqa: F401  (bass must import before tile)
import concourse.mybir as mybir
import concourse.tile as tile
from concourse import bacc, bass_isa

B, N, M, D = 4, 8192, 8192, 128
NCORES = 8
HALF = N // 2          # rows per phase-1 core
NT = HALF // 128       # 32 n-tiles per core
CW = 512               # row-side chunk width (phase-2 recompute width)
NCHUNK = M // CW       # 16 chunks per row
WD = 2624              # colacc columns on DVE; Pool handles [WD, M)
PAD = 640              # phase-2 rows per chunk-group (mean 512, sigma ~22)
GRP = NCHUNK // 2      # 8 chunk-groups per phase-2 core
NST = GRP * PAD // 128  # 40 phase-2 sub-tiles


def _build1():
    f32 = mybir.dt.float32
    f32r = mybir.dt.float32r
    nc = bacc.Bacc("TRN2", target_bir_lowering=False, debug=False,
                   num_devices=NCORES)
    at = nc.dram_tensor("at", [D, HALF], f32, kind="ExternalInput").ap()
    bt = nc.dram_tensor("bt", [D, M], f32, kind="ExternalInput").ap()
    cm_o = nc.dram_tensor("cm", [128, NT * NCHUNK], f32,
                          kind="ExternalOutput").ap()
    # rows 0..NT-1: per-tile Pool column partials (cols [WD, M) valid);
    # row NT: colacc final (cols [0, WD) valid)
    colp_o = nc.dram_tensor("colp", [NT + 1, M], f32,
                            kind="ExternalOutput").ap()

    with tile.TileContext(nc) as tc:
        with tc.tile_pool(name="big", bufs=1) as big, \
             tc.tile_pool(name="rows", bufs=2) as rows, \
             tc.tile_pool(name="cps", bufs=2) as cps, \
             tc.tile_pool(name="dmy", bufs=2) as dmy, \
             tc.tile_pool(name="ps", bufs=2, space="PSUM") as ps:
            atb = big.tile([128, HALF], f32r, name="atb")
            btb = big.tile([128, M], f32r, name="btb")
            # tile 0 needs at[:, 0:128] and then bt chunks in matmul order;
            # front-load tiny slices of those so the PE starts ASAP
            nc.sync.dma_start(atb[:, 0:128], at[:, 0:128].bitcast(f32r))
            nc.sync.dma_start(btb[:, 0:512], bt[:, 0:512].bitcast(f32r))
            nc.sync.dma_start(btb[:, 512:1024], bt[:, 512:1024].bitcast(f32r))
            for c in range(1024, M, 1024):
                nc.sync.dma_start(btb[:, c:c + 1024],
                                  bt[:, c:c + 1024].bitcast(f32r))
            nc.sync.dma_start(atb[:, 128:1024], at[:, 128:1024].bitcast(f32r))
            for c in range(1024, HALF, 1024):
                nc.sync.dma_start(atb[:, c:c + 1024],
                                  at[:, c:c + 1024].bitcast(f32r))
            cm_all = big.tile([128, NT * NCHUNK], f32, name="cm_all")
            colacc = big.tile([128, WD], f32, name="colacc")
            for t in range(NT):
                row = rows.tile([128, M], f32, tag="row", name="row")
                for c in range(4):
                    pt = ps.tile([128, 2048], f32, tag="pt", name="pt")
                    for j in range(4):
                        mlo = c * 2048 + j * 512
                        nc.tensor.matmul(pt[:, j * 512:(j + 1) * 512],
                                         atb[:, t * 128:(t + 1) * 128],
                                         btb[:, mlo:mlo + 512],
                                         start=True, stop=True)
                    nc.scalar.copy(row[:, c * 2048:(c + 1) * 2048], pt[:])
                # row side: chunk maxima via identity tensor_scalar with
                # max-accumulator (2x_2p: all-SBUF operands)
                for c in range(NCHUNK):
                    dummy = dmy.tile([128, CW], f32, tag="dmy", name="dmy")
                    nc.vector.tensor_scalar(
                        dummy[:], row[:, c * CW:(c + 1) * CW], 1.0, None,
                        op0=mybir.AluOpType.mult, op1=mybir.AluOpType.max,
                        accum_out=cm_all[:, t * NCHUNK + c:t * NCHUNK + c + 1])
                # column side, DVE part (skipped for the last tile: its
                # whole row goes through the Pool partial so the colacc
                # finalization can overlap tile NT-1)
                if t == 0:
                    nc.vector.tensor_copy(colacc[:], row[:, 0:WD])
                elif t < NT - 1:
                    nc.vector.tensor_tensor(colacc[:], colacc[:],
                                            row[:, 0:WD],
                                            op=mybir.AluOpType.max)
                # column side, Pool part: software partition reduce into
                # a separate buffer (keeps rows free of WAR/DMA holds).
                # Last tile: per-2048-chunk so each partial starts right
                # after its ACT copy, shortening the drain.
                if t < NT - 1:
                    cp = cps.tile([128, M - WD], f32, tag="cp", name="cp")
                    nc.gpsimd.partition_all_reduce(
                        cp[:], row[:, WD:M], channels=128,
                        reduce_op=bass_isa.ReduceOp.max)
                    nc.sync.dma_start(colp_o[t:t + 1, WD:M], cp[0:1, :])
                else:
                    for c in range(4):
                        lo, hi = c * 2048, (c + 1) * 2048
                        cp = cps.tile([128, 2048], f32, tag="cpl", name="cpl")
                        nc.gpsimd.partition_all_reduce(
                            cp[:], row[:, lo:hi], channels=128,
                            reduce_op=bass_isa.ReduceOp.max)
                        nc.sync.dma_start(colp_o[t:t + 1, lo:hi], cp[0:1, :])
            nc.gpsimd.partition_all_reduce(colacc[:], colacc[:], channels=128,
                                           reduce_op=bass_isa.ReduceOp.max)
            nc.sync.dma_start(colp_o[NT:NT + 1, 0:WD], colacc[0:1, :])
            nc.sync.dma_start(cm_o[:, 0:(NT - 1) * NCHUNK],
                              cm_all[:, 0:(NT - 1) * NCHUNK])
            nc.sync.dma_start(cm_o[:, (NT - 1) * NCHUNK:],
                              cm_all[:, (NT - 1) * NCHUNK:])
    nc.compile()
    return nc


def _build2():
    f32, f32r, u32 = mybir.dt.float32, mybir.dt.float32r, mybir.dt.uint32
    nc = bacc.Bacc("TRN2", target_bir_lowering=False, debug=False,
                   num_devices=NCORES)
    at2 = nc.dram_tensor("at2", [D, GRP * PAD], f32, kind="ExternalInput").ap()
    bt2 = nc.dram_tensor("bt2", [D, M // 2], f32, kind="ExternalInput").ap()
    sg = nc.dram_tensor("sg", [128, NST * 8], f32, kind="ExternalInput").ap()
    idx_o = nc.dram_tensor("idx", [128, NST * 8], u32, kind="ExternalOutput").ap()
    with tile.TileContext(nc) as tc:
        with tc.tile_pool(name="big", bufs=1) as big, \
             tc.tile_pool(name="ps", bufs=4, space="PSUM") as ps:
            a2b = big.tile([128, GRP * PAD], f32r, name="a2b")
            b2b = big.tile([128, M // 2], f32r, name="b2b")
            sgb = big.tile([128, NST * 8], f32, name="sgb")
            nc.sync.dma_start(a2b[:, 0:128], at2[:, 0:128].bitcast(f32r))
            nc.sync.dma_start(b2b[:, 0:512], bt2[:, 0:512].bitcast(f32r))
            nc.sync.dma_start(sgb[:], sg[:])
            # interleave so group 0's matmuls start before all input lands
            na = (GRP * PAD + 1023) // 1024
            nb = (M // 2) // 1024
            for i in range(max(na, nb)):
                if i < na:
                    c = i * 1024
                    lo = 128 if i == 0 else 0
                    w = min(1024, GRP * PAD - c)
                    nc.sync.dma_start(a2b[:, c + lo:c + w],
                                      at2[:, c + lo:c + w].bitcast(f32r))
                if i < nb:
                    c = i * 1024
                    lo = 512 if i == 0 else 0
                    nc.sync.dma_start(b2b[:, c + lo:c + 1024],
                                      bt2[:, c + lo:c + 1024].bitcast(f32r))
            idx8 = big.tile([128, NST * 8], u32, name="idx8")
            KP = PAD // 128
            for g in range(GRP):
                for k in range(KP):
                    st = g * KP + k
                    pt = ps.tile([128, CW], f32, tag="pt", name="pt")
                    nc.tensor.matmul(pt[:],
                                     a2b[:, st * 128:(st + 1) * 128],
                                     b2b[:, g * CW:(g + 1) * CW],
                                     start=True, stop=True)
                    nc.vector.max_index(idx8[:, st * 8:(st + 1) * 8],
                                        sgb[:, st * 8:(st + 1) * 8], pt[:])
            nc.sync.dma_start(idx_o[:], idx8[:])
    nc.compile()
    return nc


_cached = None


def _make_exec(nc):
    import jax
    from jax.sharding import Mesh, PartitionSpec
    from jax.experimental.shard_map import shard_map
    from concourse import bass2jax
    from concourse.bass2jax import _bass_exec_p

    partition_name = nc.partition_id_tensor.name if nc.partition_id_tensor else None
    in_names, out_names, out_avals, out_shapes = [], [], [], []
    for alloc in nc.m.functions[0].allocations:
        if not isinstance(alloc, mybir.MemoryLocationSet):
            continue
        name = alloc.memorylocations[0].name
        if alloc.kind == "ExternalInput":
            if name != partition_name:
                in_names.append(name)
        elif alloc.kind == "ExternalOutput":
            shape = tuple(alloc.tensor_shape)
            dtype = mybir.dt.np(alloc.dtype)
            out_names.append(name)
            out_shapes.append((shape, dtype))
            out_avals.append(jax.core.ShapedArray(shape, dtype))
    n_params = len(in_names)
    n_outs = len(out_names)
    all_in_names = in_names + out_names
    if partition_name is not None:
        all_in_names = all_in_names + [partition_name]

    def _body(*args):
        operands = list(args)
        if partition_name is not None:
            operands.append(bass2jax.partition_id_tensor())
        outs = _bass_exec_p.bind(
            *operands, out_avals=tuple(out_avals), in_names=tuple(all_in_names),
            out_names=tuple(out_names), lowering_input_output_aliases=(),
            sim_require_finite=True, sim_require_nnan=True, nc=nc)
        return tuple(outs)

    devices = jax.devices()[:NCORES]
    mesh = Mesh(np.asarray(devices), ("core",))
    in_specs = (PartitionSpec("core"),) * (n_params + n_outs)
    out_specs = (PartitionSpec("core"),) * n_outs
    fn = jax.jit(shard_map(_body, mesh=mesh, in_specs=in_specs,
                           out_specs=out_specs, check_rep=False),
                 keep_unused=True)
    return {"fn": fn, "in_names": in_names, "out_names": out_names,
            "out_shapes": out_shapes, "nc": nc}


def _run(ex, ins):
    """ins: dict name -> [NCORES, *shape]; returns dict name -> [NCORES, *shape]."""
    concat_in = [np.ascontiguousarray(ins[n].reshape(-1, *ins[n].shape[2:]))
                 for n in ex["in_names"]]
    concat_zeros = [np.zeros((NCORES * s[0], *s[1:]), dt)
                    for (s, dt) in ex["out_shapes"]]
    out_arrs = ex["fn"](*concat_in, *concat_zeros)
    return {name: np.asarray(out_arrs[i]).reshape(NCORES, *ex["out_shapes"][i][0])
            for i, name in enumerate(ex["out_names"])}


def kernel(desc0, desc1):
    global _cached
    desc0 = np.asarray(desc0, dtype=np.float32)
    desc1 = np.asarray(desc1, dtype=np.float32)
    assert desc0.shape == (B, N, D) and desc1.shape == (B, M, D)

    if _cached is None:
        _cached = (_make_exec(_build1()), _make_exec(_build2()))
    ex1, ex2 = _cached

    a_slab = np.stack([desc0[b, h * HALF:(h + 1) * HALF]
                       for b in range(B) for h in range(2)])      # [8,4096,128]
    bt_all = np.stack([desc1[b].transpose(1, 0)
                       for b in range(B) for h in range(2)])      # [8,128,8192]
    at_all = a_slab.transpose(0, 2, 1)                            # [8,128,4096]

    r1 = _run(ex1, {"at": at_all, "bt": bt_all})

    # host glue: score/chunk-argmax + grouping for phase 2
    cm = r1["cm"].reshape(NCORES, 128, NT, NCHUNK).transpose(0, 2, 1, 3) \
                 .reshape(NCORES, HALF, NCHUNK)
    cm_b = cm.reshape(B, N, NCHUNK)
    score0 = cm_b.max(axis=2)                                     # [B, N]
    cstar = cm_b.argmax(axis=2)                                   # [B, N]
    colp = r1["colp"].reshape(B, 2, NT + 1, M)
    colmax = np.empty((B, M), np.float32)
    colmax[:, :WD] = np.maximum(colp[:, :, NT, :WD],
                                colp[:, :, NT - 1, :WD]).max(axis=1)
    colmax[:, WD:] = colp[:, :, :NT, WD:].max(axis=(1, 2))

    at2 = np.zeros((NCORES, D, GRP * PAD), np.float32)
    sg = np.full((NCORES, 128, NST), 1e30, np.float32)
    slot_of_row = np.full((B, N), -1, np.int64)
    core_of_row = np.full((B, N), 0, np.int64)
    overflow = []                                                 # (b, n)
    for b in range(B):
        for g in range(NCHUNK):
            rws = np.nonzero(cstar[b] == g)[0]
            core = 2 * b + (g >= GRP)
            gl = g % GRP                                          # local group
            if len(rws) > PAD:
                overflow.extend((b, n) for n in rws[PAD:])
                rws = rws[:PAD]
            slots = gl * PAD + np.arange(len(rws))
            slot_of_row[b, rws] = slots
            core_of_row[b, rws] = core
            at2[core][:, slots] = desc0[b, rws].T
            sg[core][slots % 128, slots // 128] = score0[b, rws]

    bt2_all = np.stack([desc1[b].T[:, h2 * (M // 2):(h2 + 1) * (M // 2)]
                        for b in range(B) for h2 in range(2)])    # [8,128,4096]
    sg8 = np.repeat(sg, 8, axis=2)                         # [8,128,NST*8]
    r2 = _run(ex2, {"at2": at2, "bt2": bt2_all, "sg": sg8})
    within = r2["idx"][:, :, ::8]                                 # [8, 128, NST]

    sl = np.maximum(slot_of_row, 0)
    cr = core_of_row
    w = within[cr, sl % 128, sl // 128].astype(np.int64)          # [B, N]
    match01 = (cstar * CW + w).astype(np.int32)
    valid = (score0 > 0.1) & \
            (score0 == np.take_along_axis(colmax, match01.astype(np.int64),
                                          axis=1))

    for b, n in overflow:                                         # ~never taken
        simrow = desc0[b, n] @ desc1[b].T
        j = int(simrow.argmax())
        s = simrow.max()
        col = desc0[b] @ desc1[b, j]
        match01[b, n] = j
        score0[b, n] = np.float32(s)
        valid[b, n] = (s > 0.1) & (int(col.argmax()) == n)

    return match01, score0.astype(np.float32), valid


# revision 39
# speedup vs baseline: 2.1243x; 1.0013x over previous
"""Trainium2 Bass kernel for DescriptorMatcher (mutual nearest neighbor matching).

Problem: given desc0 [B,N,D], desc1 [B,M,D] (B=4, N=M=8192, D=128, fp32):
    sim     = desc0 @ desc1^T                      [B,N,M]
    score0  = max_m sim                            [B,N]
    match01 = argmax_m sim                         [B,N]
    match10 = argmax_n sim                         [B,M]
    valid   = (match10[match01[n]] == n) & (score0 > 0.1)
returns (match01, score0, valid).

Key reformulation: the mutual check never needs match10 indices:
    match10[match01[n]] == n  <=>  score0[n] == colmax[match01[n]]
(max chains over the same on-device fp32 values are exact).

Matmuls run in fp32r (full PE rate; ~1.6e-4 rel rounding). All downstream
max/argmax chains compare the SAME on-device fp32 PSUM values, so the
equality trick and phase-1/phase-2 consistency hold bit-exactly; only
match01-vs-fp32-reference flips remain (~0.03% of rows, well under the
2e-2 gate).

Sharding: 8 cores = 4 batches x 2 row-halves (phase 1), then
4 batches x 2 column-halves (phase 2).

Phase 1 (per core), per 128-row tile [128 x 8192]:
    PE:   16 fp32r matmuls -> PSUM [128,2048] x4
    ACT:  copy PSUM -> SBUF row buffer (fp32)
    DVE:  16 tensor_scalar(identity, accum_out=max) ops -> CM chunk maxima
          (2x_2p mode: 0.5 cyc/elem) + colacc = max(colacc, row[:, :WD])
    Pool: tensor_reduce(axis=C) on row[:, WD:] -> per-tile column partials
          (software partition reduce, ~1.44 ns/col), DMA'd per tile
  tail: one axis-C reduce of colacc -> exact column max for cols [0, WD).
  Host: score0 = CM.max, c* = CM.argmax (first occurrence), colmax from
  colacc-final + per-tile partials.

Phase 2 (per core): rows of batch b whose winning 512-wide chunk lies in
column-half h2, grouped by chunk; recompute sim[:, chunk] with
identically-laid-out fp32r matmuls (bit-exact per element), then
max_index(score, chunk) gives the exact first-occurrence within-chunk
position. match01 = chunk*512 + within.

Rows overflowing a group's padded capacity (needs >640 of ~512 expected
rows sharing one winning chunk; ~6 sigma) fall back to a host recompute.
"""

import numpy as np

import concourse.bass as bass  # noqa: F401  (bass must import before tile)
import concourse.mybir as mybir
import concourse.tile as tile
from concourse import bacc, bass_isa

B, N, M, D = 4, 8192, 8192, 128
NCORES = 8
HALF = N // 2          # rows per phase-1 core
NT = HALF // 128       # 32 n-tiles per core
CW = 512               # row-side chunk width (phase-2 recompute width)
NCHUNK = M // CW       # 16 chunks per row
WD = 2624              # colacc columns on DVE; Pool handles [WD, M)
PAD = 640              # phase-2 rows per chunk-group (mean 512, sigma ~22)
GRP = NCHUNK // 2      # 8 chunk-groups per phase-2 core
NST = GRP * PAD // 128  # 40 phase-2 sub-tiles


def _build1():
    f32 = mybir.dt.float32
    f32r = mybir.dt.float32r
    nc = bacc.Bacc("TRN2", target_bir_lowering=False, debug=False,
                   num_devices=NCORES)
    at = nc.dram_tensor("at", [D, HALF], f32, kind="ExternalInput").ap()
    bt = nc.dram_tensor("bt", [D, M], f32, kind="ExternalInput").ap()
    cm_o = nc.dram_tensor("cm", [128, NT * NCHUNK], f32,
                          kind="ExternalOutput").ap()
    # rows 0..NT-1: per-tile Pool column partials (cols [WD, M) valid);
    # row NT: colacc final (cols [0, WD) valid)
    colp_o = nc.dram_tensor("colp", [NT + 1, M], f32,
                            kind="ExternalOutput").ap()

    with tile.TileContext(nc) as tc:
        with tc.tile_pool(name="big", bufs=1) as big, \
             tc.tile_pool(name="rows", bufs=2) as rows, \
             tc.tile_pool(name="cps", bufs=2) as cps, \
             tc.tile_pool(name="dmy", bufs=2) as dmy, \
             tc.tile_pool(name="ps", bufs=2, space="PSUM") as ps:
            atb = big.tile([128, HALF], f32r, name="atb")
            btb = big.tile([128, M], f32r, name="btb")
            # tile 0 needs at[:, 0:128] and then bt chunks in matmul order;
            # front-load tiny slices of those so the PE starts ASAP
            nc.sync.dma_start(atb[:, 0:128], at[:, 0:128].bitcast(f32r))
            nc.sync.dma_start(btb[:, 0:512], bt[:, 0:512].bitcast(f32r))
            nc.sync.dma_start(btb[:, 512:1024], bt[:, 512:1024].bitcast(f32r))
            for c in range(1024, M, 1024):
                nc.sync.dma_start(btb[:, c:c + 1024],
                                  bt[:, c:c + 1024].bitcast(f32r))
            nc.sync.dma_start(atb[:, 128:1024], at[:, 128:1024].bitcast(f32r))
            for c in range(1024, HALF, 1024):
                nc.sync.dma_start(atb[:, c:c + 1024],
                                  at[:, c:c + 1024].bitcast(f32r))
            cm_all = big.tile([128, NT * NCHUNK], f32, name="cm_all")
            colacc = big.tile([128, WD], f32, name="colacc")
            for t in range(NT):
                row = rows.tile([128, M], f32, tag="row", name="row")
                for c in range(4):
                    pt = ps.tile([128, 2048], f32, tag="pt", name="pt")
                    for j in range(4):
                        mlo = c * 2048 + j * 512
                        nc.tensor.matmul(pt[:, j * 512:(j + 1) * 512],
                                         atb[:, t * 128:(t + 1) * 128],
                                         btb[:, mlo:mlo + 512],
                                         start=True, stop=True)
                    if t == 0 and c == 0:
                        # 4 narrow copies so the first DVE accum can start
                        # right after the first matmul lands
                        for j in range(4):
                            nc.scalar.copy(row[:, j * 512:(j + 1) * 512],
                                           pt[:, j * 512:(j + 1) * 512])
                    else:
                        nc.scalar.copy(row[:, c * 2048:(c + 1) * 2048], pt[:])
                # row side: chunk maxima via identity tensor_scalar with
                # max-accumulator (2x_2p: all-SBUF operands)
                for c in range(NCHUNK):
                    dummy = dmy.tile([128, CW], f32, tag="dmy", name="dmy")
                    nc.vector.tensor_scalar(
                        dummy[:], row[:, c * CW:(c + 1) * CW], 1.0, None,
                        op0=mybir.AluOpType.mult, op1=mybir.AluOpType.max,
                        accum_out=cm_all[:, t * NCHUNK + c:t * NCHUNK + c + 1])
                # column side, DVE part (skipped for the last tile: its
                # whole row goes through the Pool partial so the colacc
                # finalization can overlap tile NT-1)
                if t == 0:
                    nc.vector.tensor_copy(colacc[:], row[:, 0:WD])
                elif t < NT - 1:
                    nc.vector.tensor_tensor(colacc[:], colacc[:],
                                            row[:, 0:WD],
                                            op=mybir.AluOpType.max)
                # column side, Pool part: software partition reduce into
                # a separate buffer (keeps rows free of WAR/DMA holds).
                # Last tile: per-2048-chunk so each partial starts right
                # after its ACT copy, shortening the drain.
                if t < NT - 1:
                    cp = cps.tile([128, M - WD], f32, tag="cp", name="cp")
                    nc.gpsimd.partition_all_reduce(
                        cp[:], row[:, WD:M], channels=128,
                        reduce_op=bass_isa.ReduceOp.max)
                    nc.sync.dma_start(colp_o[t:t + 1, WD:M], cp[0:1, :])
                else:
                    for c in range(4):
                        lo, hi = c * 2048, (c + 1) * 2048
                        cp = cps.tile([128, 2048], f32, tag="cpl", name="cpl")
                        nc.gpsimd.partition_all_reduce(
                            cp[:], row[:, lo:hi], channels=128,
                            reduce_op=bass_isa.ReduceOp.max)
                        nc.sync.dma_start(colp_o[t:t + 1, lo:hi], cp[0:1, :])
            nc.gpsimd.partition_all_reduce(colacc[:], colacc[:], channels=128,
                                           reduce_op=bass_isa.ReduceOp.max)
            nc.sync.dma_start(colp_o[NT:NT + 1, 0:WD], colacc[0:1, :])
            nc.sync.dma_start(cm_o[:, 0:(NT - 1) * NCHUNK],
                              cm_all[:, 0:(NT - 1) * NCHUNK])
            nc.sync.dma_start(cm_o[:, (NT - 1) * NCHUNK:],
                              cm_all[:, (NT - 1) * NCHUNK:])
    nc.compile()
    return nc


def _build2():
    f32, f32r, u32 = mybir.dt.float32, mybir.dt.float32r, mybir.dt.uint32
    nc = bacc.Bacc("TRN2", target_bir_lowering=False, debug=False,
                   num_devices=NCORES)
    at2 = nc.dram_tensor("at2", [D, GRP * PAD], f32, kind="ExternalInput").ap()
    bt2 = nc.dram_tensor("bt2", [D, M // 2], f32, kind="ExternalInput").ap()
    sg = nc.dram_tensor("sg", [128, NST * 8], f32, kind="ExternalInput").ap()
    idx_o = nc.dram_tensor("idx", [128, NST * 8], u32, kind="ExternalOutput").ap()
    with tile.TileContext(nc) as tc:
        with tc.tile_pool(name="big", bufs=1) as big, \
             tc.tile_pool(name="ps", bufs=4, space="PSUM") as ps:
            a2b = big.tile([128, GRP * PAD], f32r, name="a2b")
            b2b = big.tile([128, M // 2], f32r, name="b2b")
            sgb = big.tile([128, NST * 8], f32, name="sgb")
            nc.sync.dma_start(a2b[:, 0:128], at2[:, 0:128].bitcast(f32r))
            nc.sync.dma_start(b2b[:, 0:512], bt2[:, 0:512].bitcast(f32r))
            nc.sync.dma_start(sgb[:], sg[:])
            # interleave so group 0's matmuls start before all input lands
            na = (GRP * PAD + 1023) // 1024
            nb = (M // 2) // 1024
            for i in range(max(na, nb)):
                if i < na:
                    c = i * 1024
                    lo = 128 if i == 0 else 0
                    w = min(1024, GRP * PAD - c)
                    nc.sync.dma_start(a2b[:, c + lo:c + w],
                                      at2[:, c + lo:c + w].bitcast(f32r))
                if i < nb:
                    c = i * 1024
                    lo = 512 if i == 0 else 0
                    nc.sync.dma_start(b2b[:, c + lo:c + 1024],
                                      bt2[:, c + lo:c + 1024].bitcast(f32r))
            idx8 = big.tile([128, NST * 8], u32, name="idx8")
            KP = PAD // 128
            for g in range(GRP):
                for k in range(KP):
                    st = g * KP + k
                    pt = ps.tile([128, CW], f32, tag="pt", name="pt")
                    nc.tensor.matmul(pt[:],
                                     a2b[:, st * 128:(st + 1) * 128],
                                     b2b[:, g * CW:(g + 1) * CW],
                                     start=True, stop=True)
                    nc.vector.max_index(idx8[:, st * 8:(st + 1) * 8],
                                        sgb[:, st * 8:(st + 1) * 8], pt[:])
            nc.sync.dma_start(idx_o[:, 0:(NST - 1) * 8],
                              idx8[:, 0:(NST - 1) * 8])
            nc.sync.dma_start(idx_o[:, (NST - 1) * 8:],
                              idx8[:, (NST - 1) * 8:])
    nc.compile()
    return nc


_cached = None


def _make_exec(nc):
    import jax
    from jax.sharding import Mesh, PartitionSpec
    from jax.experimental.shard_map import shard_map
    from concourse import bass2jax
    from concourse.bass2jax import _bass_exec_p

    partition_name = nc.partition_id_tensor.name if nc.partition_id_tensor else None
    in_names, out_names, out_avals, out_shapes = [], [], [], []
    for alloc in nc.m.functions[0].allocations:
        if not isinstance(alloc, mybir.MemoryLocationSet):
            continue
        name = alloc.memorylocations[0].name
        if alloc.kind == "ExternalInput":
            if name != partition_name:
                in_names.append(name)
        elif alloc.kind == "ExternalOutput":
            shape = tuple(alloc.tensor_shape)
            dtype = mybir.dt.np(alloc.dtype)
            out_names.append(name)
            out_shapes.append((shape, dtype))
            out_avals.append(jax.core.ShapedArray(shape, dtype))
    n_params = len(in_names)
    n_outs = len(out_names)
    all_in_names = in_names + out_names
    if partition_name is not None:
        all_in_names = all_in_names + [partition_name]

    def _body(*args):
        operands = list(args)
        if partition_name is not None:
            operands.append(bass2jax.partition_id_tensor())
        outs = _bass_exec_p.bind(
            *operands, out_avals=tuple(out_avals), in_names=tuple(all_in_names),
            out_names=tuple(out_names), lowering_input_output_aliases=(),
            sim_require_finite=True, sim_require_nnan=True, nc=nc)
        return tuple(outs)

    devices = jax.devices()[:NCORES]
    mesh = Mesh(np.asarray(devices), ("core",))
    in_specs = (PartitionSpec("core"),) * (n_params + n_outs)
    out_specs = (PartitionSpec("core"),) * n_outs
    fn = jax.jit(shard_map(_body, mesh=mesh, in_specs=in_specs,
                           out_specs=out_specs, check_rep=False),
                 keep_unused=True)
    return {"fn": fn, "in_names": in_names, "out_names": out_names,
            "out_shapes": out_shapes, "nc": nc}


def _run(ex, ins):
    """ins: dict name -> [NCORES, *shape]; returns dict name -> [NCORES, *shape]."""
    concat_in = [np.ascontiguousarray(ins[n].reshape(-1, *ins[n].shape[2:]))
                 for n in ex["in_names"]]
    concat_zeros = [np.zeros((NCORES * s[0], *s[1:]), dt)
                    for (s, dt) in ex["out_shapes"]]
    out_arrs = ex["fn"](*concat_in, *concat_zeros)
    return {name: np.asarray(out_arrs[i]).reshape(NCORES, *ex["out_shapes"][i][0])
            for i, name in enumerate(ex["out_names"])}


def kernel(desc0, desc1):
    global _cached
    desc0 = np.asarray(desc0, dtype=np.float32)
    desc1 = np.asarray(desc1, dtype=np.float32)
    assert desc0.shape == (B, N, D) and desc1.shape == (B, M, D)

    if _cached is None:
        _cached = (_make_exec(_build1()), _make_exec(_build2()))
    ex1, ex2 = _cached

    a_slab = np.stack([desc0[b, h * HALF:(h + 1) * HALF]
                       for b in range(B) for h in range(2)])      # [8,4096,128]
    bt_all = np.stack([desc1[b].transpose(1, 0)
                       for b in range(B) for h in range(2)])      # [8,128,8192]
    at_all = a_slab.transpose(0, 2, 1)                            # [8,128,4096]

    r1 = _run(ex1, {"at": at_all, "bt": bt_all})

    # host glue: score/chunk-argmax + grouping for phase 2
    cm = r1["cm"].reshape(NCORES, 128, NT, NCHUNK).transpose(0, 2, 1, 3) \
                 .reshape(NCORES, HALF, NCHUNK)
    cm_b = cm.reshape(B, N, NCHUNK)
    score0 = cm_b.max(axis=2)                                     # [B, N]
    cstar = cm_b.argmax(axis=2)                                   # [B, N]
    colp = r1["colp"].reshape(B, 2, NT + 1, M)
    colmax = np.empty((B, M), np.float32)
    colmax[:, :WD] = np.maximum(colp[:, :, NT, :WD],
                                colp[:, :, NT - 1, :WD]).max(axis=1)
    colmax[:, WD:] = colp[:, :, :NT, WD:].max(axis=(1, 2))

    at2 = np.zeros((NCORES, D, GRP * PAD), np.float32)
    sg = np.full((NCORES, 128, NST), 1e30, np.float32)
    slot_of_row = np.full((B, N), -1, np.int64)
    core_of_row = np.full((B, N), 0, np.int64)
    overflow = []                                                 # (b, n)
    for b in range(B):
        for g in range(NCHUNK):
            rws = np.nonzero(cstar[b] == g)[0]
            core = 2 * b + (g >= GRP)
            gl = g % GRP                                          # local group
            if len(rws) > PAD:
                overflow.extend((b, n) for n in rws[PAD:])
                rws = rws[:PAD]
            slots = gl * PAD + np.arange(len(rws))
            slot_of_row[b, rws] = slots
            core_of_row[b, rws] = core
            at2[core][:, slots] = desc0[b, rws].T
            sg[core][slots % 128, slots // 128] = score0[b, rws]

    bt2_all = np.stack([desc1[b].T[:, h2 * (M // 2):(h2 + 1) * (M // 2)]
                        for b in range(B) for h2 in range(2)])    # [8,128,4096]
    sg8 = np.repeat(sg, 8, axis=2)                         # [8,128,NST*8]
    r2 = _run(ex2, {"at2": at2, "bt2": bt2_all, "sg": sg8})
    within = r2["idx"][:, :, ::8]                                 # [8, 128, NST]

    sl = np.maximum(slot_of_row, 0)
    cr = core_of_row
    w = within[cr, sl % 128, sl // 128].astype(np.int64)          # [B, N]
    match01 = (cstar * CW + w).astype(np.int32)
    valid = (score0 > 0.1) & \
            (score0 == np.take_along_axis(colmax, match01.astype(np.int64),
                                          axis=1))

    for b, n in overflow:                                         # ~never taken
        simrow = desc0[b, n] @ desc1[b].T
        j = int(simrow.argmax())
        s = simrow.max()
        col = desc0[b] @ desc1[b, j]
        match01[b, n] = j
        score0[b, n] = np.float32(s)
        valid[b, n] = (s > 0.1) & (int(col.argmax()) == n)

    return match01, score0.astype(np.float32), valid


# revision 41
# speedup vs baseline: 2.1369x; 1.0059x over previous
"""Trainium2 Bass kernel for DescriptorMatcher (mutual nearest neighbor matching).

Problem: given desc0 [B,N,D], desc1 [B,M,D] (B=4, N=M=8192, D=128, fp32):
    sim     = desc0 @ desc1^T                      [B,N,M]
    score0  = max_m sim                            [B,N]
    match01 = argmax_m sim                         [B,N]
    match10 = argmax_n sim                         [B,M]
    valid   = (match10[match01[n]] == n) & (score0 > 0.1)
returns (match01, score0, valid).

Key reformulation: the mutual check never needs match10 indices:
    match10[match01[n]] == n  <=>  score0[n] == colmax[match01[n]]
(max chains over the same on-device fp32 values are exact).

Matmuls run in fp32r (full PE rate; ~1.6e-4 rel rounding). All downstream
max/argmax chains compare the SAME on-device fp32 PSUM values, so the
equality trick and phase-1/phase-2 consistency hold bit-exactly; only
match01-vs-fp32-reference flips remain (~0.03% of rows, well under the
2e-2 gate).

Sharding: 8 cores = 4 batches x 2 row-halves (phase 1), then
4 batches x 2 column-halves (phase 2).

Phase 1 (per core), per 128-row tile [128 x 8192]:
    PE:   16 fp32r matmuls -> PSUM [128,2048] x4
    ACT:  copy PSUM -> SBUF row buffer (fp32)
    DVE:  16 tensor_scalar(identity, accum_out=max) ops -> CM chunk maxima
          (2x_2p mode: 0.5 cyc/elem) + colacc = max(colacc, row[:, :WD])
    Pool: tensor_reduce(axis=C) on row[:, WD:] -> per-tile column partials
          (software partition reduce, ~1.44 ns/col), DMA'd per tile
  tail: one axis-C reduce of colacc -> exact column max for cols [0, WD).
  Host: score0 = CM.max, c* = CM.argmax (first occurrence), colmax from
  colacc-final + per-tile partials.

Phase 2 (per core): rows of batch b whose winning 512-wide chunk lies in
column-half h2, grouped by chunk; recompute sim[:, chunk] with
identically-laid-out fp32r matmuls (bit-exact per element), then
max_index(score, chunk) gives the exact first-occurrence within-chunk
position. match01 = chunk*512 + within.

Rows overflowing a group's padded capacity (needs >640 of ~512 expected
rows sharing one winning chunk; ~6 sigma) fall back to a host recompute.
"""

import numpy as np

import concourse.bass as bass  # noqa: F401  (bass must import before tile)
import concourse.mybir as mybir
import concourse.tile as tile
from concourse import bacc, bass_isa

B, N, M, D = 4, 8192, 8192, 128
NCORES = 8
HALF = N // 2          # rows per phase-1 core
NT = HALF // 128       # 32 n-tiles per core
CW = 512               # row-side chunk width (phase-2 recompute width)
NCHUNK = M // CW       # 16 chunks per row
WD = 2624              # colacc columns on DVE; Pool handles [WD, M)
PAD = 640              # phase-2 rows per chunk-group (mean 512, sigma ~22)
GRP = NCHUNK // 2      # 8 chunk-groups per phase-2 core
NST = GRP * PAD // 128  # 40 phase-2 sub-tiles


def _build1():
    f32 = mybir.dt.float32
    f32r = mybir.dt.float32r
    nc = bacc.Bacc("TRN2", target_bir_lowering=False, debug=False,
                   num_devices=NCORES)
    at = nc.dram_tensor("at", [D, HALF], f32, kind="ExternalInput").ap()
    bt = nc.dram_tensor("bt", [D, M], f32, kind="ExternalInput").ap()
    cm_o = nc.dram_tensor("cm", [128, NT * NCHUNK], f32,
                          kind="ExternalOutput").ap()
    # rows 0..NT-1: per-tile Pool column partials (cols [WD, M) valid);
    # row NT: colacc final (cols [0, WD) valid)
    colp_o = nc.dram_tensor("colp", [NT + 1, M], f32,
                            kind="ExternalOutput").ap()

    with tile.TileContext(nc) as tc:
        with tc.tile_pool(name="big", bufs=1) as big, \
             tc.tile_pool(name="rows", bufs=2) as rows, \
             tc.tile_pool(name="cps", bufs=2) as cps, \
             tc.tile_pool(name="dmy", bufs=2) as dmy, \
             tc.tile_pool(name="ps", bufs=2, space="PSUM") as ps:
            atb = big.tile([128, HALF], f32r, name="atb")
            btb = big.tile([128, M], f32r, name="btb")
            # tile 0 needs at[:, 0:128] and then bt chunks in matmul order;
            # front-load tiny slices of those so the PE starts ASAP
            nc.sync.dma_start(atb[:, 0:128], at[:, 0:128].bitcast(f32r))
            nc.sync.dma_start(btb[:, 0:512], bt[:, 0:512].bitcast(f32r))
            nc.sync.dma_start(btb[:, 512:1024], bt[:, 512:1024].bitcast(f32r))
            for c in range(1024, M, 1024):
                nc.sync.dma_start(btb[:, c:c + 1024],
                                  bt[:, c:c + 1024].bitcast(f32r))
            nc.sync.dma_start(atb[:, 128:1024], at[:, 128:1024].bitcast(f32r))
            for c in range(1024, HALF, 1024):
                nc.sync.dma_start(atb[:, c:c + 1024],
                                  at[:, c:c + 1024].bitcast(f32r))
            cm_all = big.tile([128, NT * NCHUNK], f32, name="cm_all")
            colacc = big.tile([128, WD], f32, name="colacc")
            for t in range(NT):
                row = rows.tile([128, M], f32, tag="row", name="row")
                for c in range(4):
                    pt = ps.tile([128, 2048], f32, tag="pt", name="pt")
                    for j in range(4):
                        mlo = c * 2048 + j * 512
                        nc.tensor.matmul(pt[:, j * 512:(j + 1) * 512],
                                         atb[:, t * 128:(t + 1) * 128],
                                         btb[:, mlo:mlo + 512],
                                         start=True, stop=True)
                    if t == 0 and c == 0:
                        # 4 narrow copies so the first DVE accum can start
                        # right after the first matmul lands
                        for j in range(4):
                            nc.scalar.copy(row[:, j * 512:(j + 1) * 512],
                                           pt[:, j * 512:(j + 1) * 512])
                    else:
                        nc.scalar.copy(row[:, c * 2048:(c + 1) * 2048], pt[:])
                # row side: chunk maxima via identity tensor_scalar with
                # max-accumulator (2x_2p: all-SBUF operands)
                for c in range(NCHUNK):
                    dummy = dmy.tile([128, CW], f32, tag="dmy", name="dmy")
                    nc.vector.tensor_scalar(
                        dummy[:], row[:, c * CW:(c + 1) * CW], 1.0, None,
                        op0=mybir.AluOpType.mult, op1=mybir.AluOpType.max,
                        accum_out=cm_all[:, t * NCHUNK + c:t * NCHUNK + c + 1])
                # column side, DVE part (skipped for the last tile: its
                # whole row goes through the Pool partial so the colacc
                # finalization can overlap tile NT-1)
                if t == 0:
                    nc.vector.tensor_copy(colacc[:], row[:, 0:WD])
                elif t < NT - 1:
                    nc.vector.tensor_tensor(colacc[:], colacc[:],
                                            row[:, 0:WD],
                                            op=mybir.AluOpType.max)
                # column side, Pool part: software partition reduce into
                # a separate buffer (keeps rows free of WAR/DMA holds).
                # Last tile: per-2048-chunk so each partial starts right
                # after its ACT copy, shortening the drain.
                if t < 2:
                    for lo, hi in ((WD, 4096), (4096, 6144), (6144, M)):
                        cp = cps.tile([128, hi - lo], f32, tag="cpl",
                                      name="cpe")
                        nc.gpsimd.partition_all_reduce(
                            cp[:], row[:, lo:hi], channels=128,
                            reduce_op=bass_isa.ReduceOp.max)
                        nc.sync.dma_start(colp_o[t:t + 1, lo:hi], cp[0:1, :])
                elif t < NT - 1:
                    cp = cps.tile([128, M - WD], f32, tag="cp", name="cp")
                    nc.gpsimd.partition_all_reduce(
                        cp[:], row[:, WD:M], channels=128,
                        reduce_op=bass_isa.ReduceOp.max)
                    nc.sync.dma_start(colp_o[t:t + 1, WD:M], cp[0:1, :])
                else:
                    for c in range(4):
                        lo, hi = c * 2048, (c + 1) * 2048
                        cp = cps.tile([128, 2048], f32, tag="cpl", name="cpl")
                        nc.gpsimd.partition_all_reduce(
                            cp[:], row[:, lo:hi], channels=128,
                            reduce_op=bass_isa.ReduceOp.max)
                        nc.sync.dma_start(colp_o[t:t + 1, lo:hi], cp[0:1, :])
            nc.gpsimd.partition_all_reduce(colacc[:], colacc[:], channels=128,
                                           reduce_op=bass_isa.ReduceOp.max)
            nc.sync.dma_start(colp_o[NT:NT + 1, 0:WD], colacc[0:1, :])
            nc.sync.dma_start(cm_o[:, 0:(NT - 1) * NCHUNK],
                              cm_all[:, 0:(NT - 1) * NCHUNK])
            nc.sync.dma_start(cm_o[:, (NT - 1) * NCHUNK:],
                              cm_all[:, (NT - 1) * NCHUNK:])
    nc.compile()
    return nc


def _build2():
    f32, f32r, u32 = mybir.dt.float32, mybir.dt.float32r, mybir.dt.uint32
    nc = bacc.Bacc("TRN2", target_bir_lowering=False, debug=False,
                   num_devices=NCORES)
    at2 = nc.dram_tensor("at2", [D, GRP * PAD], f32, kind="ExternalInput").ap()
    bt2 = nc.dram_tensor("bt2", [D, M // 2], f32, kind="ExternalInput").ap()
    sg = nc.dram_tensor("sg", [128, NST * 8], f32, kind="ExternalInput").ap()
    idx_o = nc.dram_tensor("idx", [128, NST * 8], u32, kind="ExternalOutput").ap()
    with tile.TileContext(nc) as tc:
        with tc.tile_pool(name="big", bufs=1) as big, \
             tc.tile_pool(name="stg", bufs=4) as stg, \
             tc.tile_pool(name="ps", bufs=4, space="PSUM") as ps:
            a2b = big.tile([128, GRP * PAD], f32r, name="a2b")
            b2b = big.tile([128, M // 2], f32r, name="b2b")
            sgb = big.tile([128, NST * 8], f32, name="sgb")
            nc.sync.dma_start(a2b[:, 0:128], at2[:, 0:128].bitcast(f32r))
            nc.sync.dma_start(b2b[:, 0:512], bt2[:, 0:512].bitcast(f32r))
            nc.sync.dma_start(sgb[:], sg[:])
            # interleave so group 0's matmuls start before all input lands
            na = (GRP * PAD + 1023) // 1024
            nb = (M // 2) // 1024
            for i in range(max(na, nb)):
                if i < na:
                    c = i * 1024
                    lo = 128 if i == 0 else 0
                    w = min(1024, GRP * PAD - c)
                    nc.sync.dma_start(a2b[:, c + lo:c + w],
                                      at2[:, c + lo:c + w].bitcast(f32r))
                if i < nb:
                    c = i * 1024
                    lo = 512 if i == 0 else 0
                    nc.sync.dma_start(b2b[:, c + lo:c + 1024],
                                      bt2[:, c + lo:c + 1024].bitcast(f32r))
            idx8 = big.tile([128, NST * 8], u32, name="idx8")
            KP = PAD // 128
            for g in range(GRP):
                for k in range(KP):
                    st = g * KP + k
                    pt = ps.tile([128, CW], f32, tag="pt", name="pt")
                    nc.tensor.matmul(pt[:],
                                     a2b[:, st * 128:(st + 1) * 128],
                                     b2b[:, g * CW:(g + 1) * CW],
                                     start=True, stop=True)
                    ch = stg.tile([128, CW], f32, tag="ch", name="ch")
                    nc.scalar.copy(ch[:], pt[:])
                    nc.vector.max_index(idx8[:, st * 8:(st + 1) * 8],
                                        sgb[:, st * 8:(st + 1) * 8], ch[:])
            nc.sync.dma_start(idx_o[:, 0:(NST - 1) * 8],
                              idx8[:, 0:(NST - 1) * 8])
            nc.sync.dma_start(idx_o[:, (NST - 1) * 8:],
                              idx8[:, (NST - 1) * 8:])
    nc.compile()
    return nc


_cached = None


def _make_exec(nc):
    import jax
    from jax.sharding import Mesh, PartitionSpec
    from jax.experimental.shard_map import shard_map
    from concourse import bass2jax
    from concourse.bass2jax import _bass_exec_p

    partition_name = nc.partition_id_tensor.name if nc.partition_id_tensor else None
    in_names, out_names, out_avals, out_shapes = [], [], [], []
    for alloc in nc.m.functions[0].allocations:
        if not isinstance(alloc, mybir.MemoryLocationSet):
            continue
        name = alloc.memorylocations[0].name
        if alloc.kind == "ExternalInput":
            if name != partition_name:
                in_names.append(name)
        elif alloc.kind == "ExternalOutput":
            shape = tuple(alloc.tensor_shape)
            dtype = mybir.dt.np(alloc.dtype)
            out_names.append(name)
            out_shapes.append((shape, dtype))
            out_avals.append(jax.core.ShapedArray(shape, dtype))
    n_params = len(in_names)
    n_outs = len(out_names)
    all_in_names = in_names + out_names
    if partition_name is not None:
        all_in_names = all_in_names + [partition_name]

    def _body(*args):
        operands = list(args)
        if partition_name is not None:
            operands.append(bass2jax.partition_id_tensor())
        outs = _bass_exec_p.bind(
            *operands, out_avals=tuple(out_avals), in_names=tuple(all_in_names),
            out_names=tuple(out_names), lowering_input_output_aliases=(),
            sim_require_finite=True, sim_require_nnan=True, nc=nc)
        return tuple(outs)

    devices = jax.devices()[:NCORES]
    mesh = Mesh(np.asarray(devices), ("core",))
    in_specs = (PartitionSpec("core"),) * (n_params + n_outs)
    out_specs = (PartitionSpec("core"),) * n_outs
    fn = jax.jit(shard_map(_body, mesh=mesh, in_specs=in_specs,
                           out_specs=out_specs, check_rep=False),
                 keep_unused=True)
    return {"fn": fn, "in_names": in_names, "out_names": out_names,
            "out_shapes": out_shapes, "nc": nc}


def _run(ex, ins):
    """ins: dict name -> [NCORES, *shape]; returns dict name -> [NCORES, *shape]."""
    concat_in = [np.ascontiguousarray(ins[n].reshape(-1, *ins[n].shape[2:]))
                 for n in ex["in_names"]]
    concat_zeros = [np.zeros((NCORES * s[0], *s[1:]), dt)
                    for (s, dt) in ex["out_shapes"]]
    out_arrs = ex["fn"](*concat_in, *concat_zeros)
    return {name: np.asarray(out_arrs[i]).reshape(NCORES, *ex["out_shapes"][i][0])
            for i, name in enumerate(ex["out_names"])}


def kernel(desc0, desc1):
    global _cached
    desc0 = np.asarray(desc0, dtype=np.float32)
    desc1 = np.asarray(desc1, dtype=np.float32)
    assert desc0.shape == (B, N, D) and desc1.shape == (B, M, D)

    if _cached is None:
        _cached = (_make_exec(_build1()), _make_exec(_build2()))
    ex1, ex2 = _cached

    a_slab = np.stack([desc0[b, h * HALF:(h + 1) * HALF]
                       for b in range(B) for h in range(2)])      # [8,4096,128]
    bt_all = np.stack([desc1[b].transpose(1, 0)
                       for b in range(B) for h in range(2)])      # [8,128,8192]
    at_all = a_slab.transpose(0, 2, 1)                            # [8,128,4096]

    r1 = _run(ex1, {"at": at_all, "bt": bt_all})

    # host glue: score/chunk-argmax + grouping for phase 2
    cm = r1["cm"].reshape(NCORES, 128, NT, NCHUNK).transpose(0, 2, 1, 3) \
                 .reshape(NCORES, HALF, NCHUNK)
    cm_b = cm.reshape(B, N, NCHUNK)
    score0 = cm_b.max(axis=2)                                     # [B, N]
    cstar = cm_b.argmax(axis=2)                                   # [B, N]
    colp = r1["colp"].reshape(B, 2, NT + 1, M)
    colmax = np.empty((B, M), np.float32)
    colmax[:, :WD] = np.maximum(colp[:, :, NT, :WD],
                                colp[:, :, NT - 1, :WD]).max(axis=1)
    colmax[:, WD:] = colp[:, :, :NT, WD:].max(axis=(1, 2))

    at2 = np.zeros((NCORES, D, GRP * PAD), np.float32)
    sg = np.full((NCORES, 128, NST), 1e30, np.float32)
    slot_of_row = np.full((B, N), -1, np.int64)
    core_of_row = np.full((B, N), 0, np.int64)
    overflow = []                                                 # (b, n)
    for b in range(B):
        for g in range(NCHUNK):
            rws = np.nonzero(cstar[b] == g)[0]
            core = 2 * b + (g >= GRP)
            gl = g % GRP                                          # local group
            if len(rws) > PAD:
                overflow.extend((b, n) for n in rws[PAD:])
                rws = rws[:PAD]
            slots = gl * PAD + np.arange(len(rws))
            slot_of_row[b, rws] = slots
            core_of_row[b, rws] = core
            at2[core][:, slots] = desc0[b, rws].T
            sg[core][slots % 128, slots // 128] = score0[b, rws]

    bt2_all = np.stack([desc1[b].T[:, h2 * (M // 2):(h2 + 1) * (M // 2)]
                        for b in range(B) for h2 in range(2)])    # [8,128,4096]
    sg8 = np.repeat(sg, 8, axis=2)                         # [8,128,NST*8]
    r2 = _run(ex2, {"at2": at2, "bt2": bt2_all, "sg": sg8})
    within = r2["idx"][:, :, ::8]                                 # [8, 128, NST]

    sl = np.maximum(slot_of_row, 0)
    cr = core_of_row
    w = within[cr, sl % 128, sl // 128].astype(np.int64)          # [B, N]
    match01 = (cstar * CW + w).astype(np.int32)
    valid = (score0 > 0.1) & \
            (score0 == np.take_along_axis(colmax, match01.astype(np.int64),
                                          axis=1))

    for b, n in overflow:                                         # ~never taken
        simrow = desc0[b, n] @ desc1[b].T
        j = int(simrow.argmax())
        s = simrow.max()
        col = desc0[b] @ desc1[b, j]
        match01[b, n] = j
        score0[b, n] = np.float32(s)
        valid[b, n] = (s > 0.1) & (int(col.argmax()) == n)

    return match01, score0.astype(np.float32), valid


# revision 44
# speedup vs baseline: 2.1569x; 1.0094x over previous
"""Trainium2 Bass kernel for DescriptorMatcher (mutual nearest neighbor matching).

Problem: given desc0 [B,N,D], desc1 [B,M,D] (B=4, N=M=8192, D=128, fp32):
    sim     = desc0 @ desc1^T                      [B,N,M]
    score0  = max_m sim                            [B,N]
    match01 = argmax_m sim                         [B,N]
    match10 = argmax_n sim                         [B,M]
    valid   = (match10[match01[n]] == n) & (score0 > 0.1)
returns (match01, score0, valid).

Key reformulation: the mutual check never needs match10 indices:
    match10[match01[n]] == n  <=>  score0[n] == colmax[match01[n]]
(max chains over the same on-device fp32 values are exact).

Matmuls run in fp32r (full PE rate; ~1.6e-4 rel rounding). All downstream
max/argmax chains compare the SAME on-device fp32 PSUM values, so the
equality trick and phase-1/phase-2 consistency hold bit-exactly; only
match01-vs-fp32-reference flips remain (~0.03% of rows, well under the
2e-2 gate).

Sharding: 8 cores = 4 batches x 2 row-halves (phase 1), then
4 batches x 2 column-halves (phase 2).

Phase 1 (per core), per 128-row tile [128 x 8192]:
    PE:   16 fp32r matmuls -> PSUM [128,2048] x4
    ACT:  copy PSUM -> SBUF row buffer (fp32)
    DVE:  16 tensor_scalar(identity, accum_out=max) ops -> CM chunk maxima
          (2x_2p mode: 0.5 cyc/elem) + colacc = max(colacc, row[:, :WD])
    Pool: tensor_reduce(axis=C) on row[:, WD:] -> per-tile column partials
          (software partition reduce, ~1.44 ns/col), DMA'd per tile
  tail: one axis-C reduce of colacc -> exact column max for cols [0, WD).
  Host: score0 = CM.max, c* = CM.argmax (first occurrence), colmax from
  colacc-final + per-tile partials.

Phase 2 (per core): rows of batch b whose winning 512-wide chunk lies in
column-half h2, grouped by chunk; recompute sim[:, chunk] with
identically-laid-out fp32r matmuls (bit-exact per element), then
max_index(score, chunk) gives the exact first-occurrence within-chunk
position. match01 = chunk*512 + within.

Rows overflowing a group's padded capacity (needs >640 of ~512 expected
rows sharing one winning chunk; ~6 sigma) fall back to a host recompute.
"""

import numpy as np

import concourse.bass as bass  # noqa: F401  (bass must import before tile)
import concourse.mybir as mybir
import concourse.tile as tile
from concourse import bacc, bass_isa

B, N, M, D = 4, 8192, 8192, 128
NCORES = 8
HALF = N // 2          # rows per phase-1 core
NT = HALF // 128       # 32 n-tiles per core
CW = 512               # row-side chunk width (phase-2 recompute width)
NCHUNK = M // CW       # 16 chunks per row
WD = 2624              # colacc columns on DVE; Pool handles [WD, M)
PAD = 640              # phase-2 rows per chunk-group (mean 512, sigma ~22)
GRP = NCHUNK // 2      # 8 chunk-groups per phase-2 core
NST = GRP * PAD // 128  # 40 phase-2 sub-tiles


def _build1():
    f32 = mybir.dt.float32
    f32r = mybir.dt.float32r
    nc = bacc.Bacc("TRN2", target_bir_lowering=False, debug=False,
                   num_devices=NCORES)
    at = nc.dram_tensor("at", [D, HALF], f32, kind="ExternalInput").ap()
    bt = nc.dram_tensor("bt", [D, M], f32, kind="ExternalInput").ap()
    cm_o = nc.dram_tensor("cm", [128, NT * NCHUNK], f32,
                          kind="ExternalOutput").ap()
    # rows 0..NT-1: per-tile Pool column partials (cols [WD, M) valid);
    # row NT: colacc final (cols [0, WD) valid)
    colp_o = nc.dram_tensor("colp", [NT + 1, M], f32,
                            kind="ExternalOutput").ap()

    with tile.TileContext(nc) as tc:
        with tc.tile_pool(name="big", bufs=1) as big, \
             tc.tile_pool(name="rows", bufs=2) as rows, \
             tc.tile_pool(name="cps", bufs=2) as cps, \
             tc.tile_pool(name="dmy", bufs=2) as dmy, \
             tc.tile_pool(name="ps", bufs=2, space="PSUM") as ps:
            atb = big.tile([128, HALF], f32r, name="atb")
            btb = big.tile([128, M], f32r, name="btb")
            # tile 0 needs at[:, 0:128] and then bt chunks in matmul order;
            # front-load tiny slices of those so the PE starts ASAP
            nc.sync.dma_start(atb[:, 0:128], at[:, 0:128].bitcast(f32r))
            nc.sync.dma_start(btb[:, 0:512], bt[:, 0:512].bitcast(f32r))
            nc.sync.dma_start(btb[:, 512:1024], bt[:, 512:1024].bitcast(f32r))
            for c in range(1024, M, 1024):
                nc.sync.dma_start(btb[:, c:c + 1024],
                                  bt[:, c:c + 1024].bitcast(f32r))
            nc.sync.dma_start(atb[:, 128:1024], at[:, 128:1024].bitcast(f32r))
            for c in range(1024, HALF, 1024):
                nc.sync.dma_start(atb[:, c:c + 1024],
                                  at[:, c:c + 1024].bitcast(f32r))
            cm_all = big.tile([128, NT * NCHUNK], f32, name="cm_all")
            colacc = big.tile([128, WD], f32, name="colacc")
            for t in range(NT):
                row = rows.tile([128, M], f32, tag="row", name="row")
                for c in range(4):
                    pt = ps.tile([128, 2048], f32, tag="pt", name="pt")
                    for j in range(4):
                        mlo = c * 2048 + j * 512
                        nc.tensor.matmul(pt[:, j * 512:(j + 1) * 512],
                                         atb[:, t * 128:(t + 1) * 128],
                                         btb[:, mlo:mlo + 512],
                                         start=True, stop=True)
                    if t == 0 and c == 0:
                        # 4 narrow copies so the first DVE accum can start
                        # right after the first matmul lands
                        for j in range(4):
                            nc.scalar.copy(row[:, j * 512:(j + 1) * 512],
                                           pt[:, j * 512:(j + 1) * 512])
                    else:
                        nc.scalar.copy(row[:, c * 2048:(c + 1) * 2048], pt[:])
                # row side: chunk maxima via identity tensor_scalar with
                # max-accumulator (2x_2p: all-SBUF operands)
                for c in range(NCHUNK):
                    dummy = dmy.tile([128, CW], f32, tag="dmy", name="dmy")
                    nc.vector.tensor_scalar(
                        dummy[:], row[:, c * CW:(c + 1) * CW], 1.0, None,
                        op0=mybir.AluOpType.mult, op1=mybir.AluOpType.max,
                        accum_out=cm_all[:, t * NCHUNK + c:t * NCHUNK + c + 1])
                # column side, DVE part
                if t == 0:
                    nc.vector.tensor_copy(colacc[:], row[:, 0:WD])
                else:
                    nc.vector.tensor_tensor(colacc[:], colacc[:],
                                            row[:, 0:WD],
                                            op=mybir.AluOpType.max)
                # column side, Pool part: software partition reduce into
                # a separate buffer (keeps rows free of WAR/DMA holds).
                # Last tile: per-2048-chunk so each partial starts right
                # after its ACT copy, shortening the drain.
                if t < 2:
                    for lo, hi in ((WD, 4096), (4096, 6144), (6144, M)):
                        cp = cps.tile([128, hi - lo], f32, tag="cpl",
                                      name="cpe")
                        nc.gpsimd.partition_all_reduce(
                            cp[:], row[:, lo:hi], channels=128,
                            reduce_op=bass_isa.ReduceOp.max)
                        nc.sync.dma_start(colp_o[t:t + 1, lo:hi], cp[0:1, :])
                else:
                    cp = cps.tile([128, M - WD], f32, tag="cp", name="cp")
                    nc.gpsimd.partition_all_reduce(
                        cp[:], row[:, WD:M], channels=128,
                        reduce_op=bass_isa.ReduceOp.max)
                    nc.sync.dma_start(colp_o[t:t + 1, WD:M], cp[0:1, :])
            nc.gpsimd.partition_all_reduce(colacc[:], colacc[:], channels=128,
                                           reduce_op=bass_isa.ReduceOp.max)
            nc.sync.dma_start(colp_o[NT:NT + 1, 0:WD], colacc[0:1, :])
            nc.sync.dma_start(cm_o[:, 0:(NT - 1) * NCHUNK],
                              cm_all[:, 0:(NT - 1) * NCHUNK])
            nc.sync.dma_start(cm_o[:, (NT - 1) * NCHUNK:],
                              cm_all[:, (NT - 1) * NCHUNK:])
    nc.compile()
    return nc


def _build2():
    f32, f32r, u32 = mybir.dt.float32, mybir.dt.float32r, mybir.dt.uint32
    nc = bacc.Bacc("TRN2", target_bir_lowering=False, debug=False,
                   num_devices=NCORES)
    at2 = nc.dram_tensor("at2", [D, GRP * PAD], f32, kind="ExternalInput").ap()
    bt2 = nc.dram_tensor("bt2", [D, M // 2], f32, kind="ExternalInput").ap()
    sg = nc.dram_tensor("sg", [128, NST * 8], f32, kind="ExternalInput").ap()
    idx_o = nc.dram_tensor("idx", [128, NST * 8], u32, kind="ExternalOutput").ap()
    with tile.TileContext(nc) as tc:
        with tc.tile_pool(name="big", bufs=1) as big, \
             tc.tile_pool(name="stg", bufs=4) as stg, \
             tc.tile_pool(name="ps", bufs=4, space="PSUM") as ps:
            a2b = big.tile([128, GRP * PAD], f32r, name="a2b")
            b2b = big.tile([128, M // 2], f32r, name="b2b")
            sgb = big.tile([128, NST * 8], f32, name="sgb")
            nc.sync.dma_start(a2b[:, 0:128], at2[:, 0:128].bitcast(f32r))
            nc.sync.dma_start(b2b[:, 0:512], bt2[:, 0:512].bitcast(f32r))
            nc.sync.dma_start(sgb[:], sg[:])
            # interleave so group 0's matmuls start before all input lands
            na = (GRP * PAD + 1023) // 1024
            nb = (M // 2) // 1024
            for i in range(max(na, nb)):
                if i < na:
                    c = i * 1024
                    lo = 128 if i == 0 else 0
                    w = min(1024, GRP * PAD - c)
                    nc.sync.dma_start(a2b[:, c + lo:c + w],
                                      at2[:, c + lo:c + w].bitcast(f32r))
                if i < nb:
                    c = i * 1024
                    lo = 512 if i == 0 else 0
                    nc.sync.dma_start(b2b[:, c + lo:c + 1024],
                                      bt2[:, c + lo:c + 1024].bitcast(f32r))
            idx8 = big.tile([128, NST * 8], u32, name="idx8")
            KP = PAD // 128
            for g in range(GRP):
                for k in range(KP):
                    st = g * KP + k
                    pt = ps.tile([128, CW], f32, tag="pt", name="pt")
                    nc.tensor.matmul(pt[:],
                                     a2b[:, st * 128:(st + 1) * 128],
                                     b2b[:, g * CW:(g + 1) * CW],
                                     start=True, stop=True)
                    ch = stg.tile([128, CW], f32, tag="ch", name="ch")
                    nc.scalar.copy(ch[:], pt[:])
                    nc.vector.max_index(idx8[:, st * 8:(st + 1) * 8],
                                        sgb[:, st * 8:(st + 1) * 8], ch[:])
            nc.sync.dma_start(idx_o[:, 0:(NST - 1) * 8],
                              idx8[:, 0:(NST - 1) * 8])
            nc.sync.dma_start(idx_o[:, (NST - 1) * 8:],
                              idx8[:, (NST - 1) * 8:])
    nc.compile()
    return nc


_cached = None


def _make_exec(nc):
    import jax
    from jax.sharding import Mesh, PartitionSpec
    from jax.experimental.shard_map import shard_map
    from concourse import bass2jax
    from concourse.bass2jax import _bass_exec_p

    partition_name = nc.partition_id_tensor.name if nc.partition_id_tensor else None
    in_names, out_names, out_avals, out_shapes = [], [], [], []
    for alloc in nc.m.functions[0].allocations:
        if not isinstance(alloc, mybir.MemoryLocationSet):
            continue
        name = alloc.memorylocations[0].name
        if alloc.kind == "ExternalInput":
            if name != partition_name:
                in_names.append(name)
        elif alloc.kind == "ExternalOutput":
            shape = tuple(alloc.tensor_shape)
            dtype = mybir.dt.np(alloc.dtype)
            out_names.append(name)
            out_shapes.append((shape, dtype))
            out_avals.append(jax.core.ShapedArray(shape, dtype))
    n_params = len(in_names)
    n_outs = len(out_names)
    all_in_names = in_names + out_names
    if partition_name is not None:
        all_in_names = all_in_names + [partition_name]

    def _body(*args):
        operands = list(args)
        if partition_name is not None:
            operands.append(bass2jax.partition_id_tensor())
        outs = _bass_exec_p.bind(
            *operands, out_avals=tuple(out_avals), in_names=tuple(all_in_names),
            out_names=tuple(out_names), lowering_input_output_aliases=(),
            sim_require_finite=True, sim_require_nnan=True, nc=nc)
        return tuple(outs)

    devices = jax.devices()[:NCORES]
    mesh = Mesh(np.asarray(devices), ("core",))
    in_specs = (PartitionSpec("core"),) * (n_params + n_outs)
    out_specs = (PartitionSpec("core"),) * n_outs
    fn = jax.jit(shard_map(_body, mesh=mesh, in_specs=in_specs,
                           out_specs=out_specs, check_rep=False),
                 keep_unused=True)
    return {"fn": fn, "in_names": in_names, "out_names": out_names,
            "out_shapes": out_shapes, "nc": nc}


def _run(ex, ins):
    """ins: dict name -> [NCORES, *shape]; returns dict name -> [NCORES, *shape]."""
    concat_in = [np.ascontiguousarray(ins[n].reshape(-1, *ins[n].shape[2:]))
                 for n in ex["in_names"]]
    concat_zeros = [np.zeros((NCORES * s[0], *s[1:]), dt)
                    for (s, dt) in ex["out_shapes"]]
    out_arrs = ex["fn"](*concat_in, *concat_zeros)
    return {name: np.asarray(out_arrs[i]).reshape(NCORES, *ex["out_shapes"][i][0])
            for i, name in enumerate(ex["out_names"])}


def kernel(desc0, desc1):
    global _cached
    desc0 = np.asarray(desc0, dtype=np.float32)
    desc1 = np.asarray(desc1, dtype=np.float32)
    assert desc0.shape == (B, N, D) and desc1.shape == (B, M, D)

    if _cached is None:
        _cached = (_make_exec(_build1()), _make_exec(_build2()))
    ex1, ex2 = _cached

    a_slab = np.stack([desc0[b, h * HALF:(h + 1) * HALF]
                       for b in range(B) for h in range(2)])      # [8,4096,128]
    bt_all = np.stack([desc1[b].transpose(1, 0)
                       for b in range(B) for h in range(2)])      # [8,128,8192]
    at_all = a_slab.transpose(0, 2, 1)                            # [8,128,4096]

    r1 = _run(ex1, {"at": at_all, "bt": bt_all})

    # host glue: score/chunk-argmax + grouping for phase 2
    cm = r1["cm"].reshape(NCORES, 128, NT, NCHUNK).transpose(0, 2, 1, 3) \
                 .reshape(NCORES, HALF, NCHUNK)
    cm_b = cm.reshape(B, N, NCHUNK)
    score0 = cm_b.max(axis=2)                                     # [B, N]
    cstar = cm_b.argmax(axis=2)                                   # [B, N]
    colp = r1["colp"].reshape(B, 2, NT + 1, M)
    colmax = np.empty((B, M), np.float32)
    colmax[:, :WD] = colp[:, :, NT, :WD].max(axis=1)
    colmax[:, WD:] = colp[:, :, :NT, WD:].max(axis=(1, 2))

    at2 = np.zeros((NCORES, D, GRP * PAD), np.float32)
    sg = np.full((NCORES, 128, NST), 1e30, np.float32)
    slot_of_row = np.full((B, N), -1, np.int64)
    core_of_row = np.full((B, N), 0, np.int64)
    overflow = []                                                 # (b, n)
    for b in range(B):
        for g in range(NCHUNK):
            rws = np.nonzero(cstar[b] == g)[0]
            core = 2 * b + (g >= GRP)
            gl = g % GRP                                          # local group
            if len(rws) > PAD:
                overflow.extend((b, n) for n in rws[PAD:])
                rws = rws[:PAD]
            slots = gl * PAD + np.arange(len(rws))
            slot_of_row[b, rws] = slots
            core_of_row[b, rws] = core
            at2[core][:, slots] = desc0[b, rws].T
            sg[core][slots % 128, slots // 128] = score0[b, rws]

    bt2_all = np.stack([desc1[b].T[:, h2 * (M // 2):(h2 + 1) * (M // 2)]
                        for b in range(B) for h2 in range(2)])    # [8,128,4096]
    sg8 = np.repeat(sg, 8, axis=2)                         # [8,128,NST*8]
    r2 = _run(ex2, {"at2": at2, "bt2": bt2_all, "sg": sg8})
    within = r2["idx"][:, :, ::8]                                 # [8, 128, NST]

    sl = np.maximum(slot_of_row, 0)
    cr = core_of_row
    w = within[cr, sl % 128, sl // 128].astype(np.int64)          # [B, N]
    match01 = (cstar * CW + w).astype(np.int32)
    valid = (score0 > 0.1) & \
            (score0 == np.take_along_axis(colmax, match01.astype(np.int64),
                                          axis=1))

    for b, n in overflow:                                         # ~never taken
        simrow = desc0[b, n] @ desc1[b].T
        j = int(simrow.argmax())
        s = simrow.max()
        col = desc0[b] @ desc1[b, j]
        match01[b, n] = j
        score0[b, n] = np.float32(s)
        valid[b, n] = (s > 0.1) & (int(col.argmax()) == n)

    return match01, score0.astype(np.float32), valid


# revision 45
# speedup vs baseline: 2.1675x; 1.0049x over previous
"""Trainium2 Bass kernel for DescriptorMatcher (mutual nearest neighbor matching).

Problem: given desc0 [B,N,D], desc1 [B,M,D] (B=4, N=M=8192, D=128, fp32):
    sim     = desc0 @ desc1^T                      [B,N,M]
    score0  = max_m sim                            [B,N]
    match01 = argmax_m sim                         [B,N]
    match10 = argmax_n sim                         [B,M]
    valid   = (match10[match01[n]] == n) & (score0 > 0.1)
returns (match01, score0, valid).

Key reformulation: the mutual check never needs match10 indices:
    match10[match01[n]] == n  <=>  score0[n] == colmax[match01[n]]
(max chains over the same on-device fp32 values are exact).

Matmuls run in fp32r (full PE rate; ~1.6e-4 rel rounding). All downstream
max/argmax chains compare the SAME on-device fp32 PSUM values, so the
equality trick and phase-1/phase-2 consistency hold bit-exactly; only
match01-vs-fp32-reference flips remain (~0.03% of rows, well under the
2e-2 gate).

Sharding: 8 cores = 4 batches x 2 row-halves (phase 1), then
4 batches x 2 column-halves (phase 2).

Phase 1 (per core), per 128-row tile [128 x 8192]:
    PE:   16 fp32r matmuls -> PSUM [128,2048] x4
    ACT:  copy PSUM -> SBUF row buffer (fp32)
    DVE:  16 tensor_scalar(identity, accum_out=max) ops -> CM chunk maxima
          (2x_2p mode: 0.5 cyc/elem) + colacc = max(colacc, row[:, :WD])
    Pool: tensor_reduce(axis=C) on row[:, WD:] -> per-tile column partials
          (software partition reduce, ~1.44 ns/col), DMA'd per tile
  tail: one axis-C reduce of colacc -> exact column max for cols [0, WD).
  Host: score0 = CM.max, c* = CM.argmax (first occurrence), colmax from
  colacc-final + per-tile partials.

Phase 2 (per core): rows of batch b whose winning 512-wide chunk lies in
column-half h2, grouped by chunk; recompute sim[:, chunk] with
identically-laid-out fp32r matmuls (bit-exact per element), then
max_index(score, chunk) gives the exact first-occurrence within-chunk
position. match01 = chunk*512 + within.

Rows overflowing a group's padded capacity (needs >640 of ~512 expected
rows sharing one winning chunk; ~6 sigma) fall back to a host recompute.
"""

import numpy as np

import concourse.bass as bass  # noqa: F401  (bass must import before tile)
import concourse.mybir as mybir
import concourse.tile as tile
from concourse import bacc, bass_isa

B, N, M, D = 4, 8192, 8192, 128
NCORES = 8
HALF = N // 2          # rows per phase-1 core
NT = HALF // 128       # 32 n-tiles per core
CW = 512               # row-side chunk width (phase-2 recompute width)
NCHUNK = M // CW       # 16 chunks per row
WD = 2560              # colacc columns on DVE; Pool handles [WD, M)
PAD = 640              # phase-2 rows per chunk-group (mean 512, sigma ~22)
GRP = NCHUNK // 2      # 8 chunk-groups per phase-2 core
NST = GRP * PAD // 128  # 40 phase-2 sub-tiles


def _build1():
    f32 = mybir.dt.float32
    f32r = mybir.dt.float32r
    nc = bacc.Bacc("TRN2", target_bir_lowering=False, debug=False,
                   num_devices=NCORES)
    at = nc.dram_tensor("at", [D, HALF], f32, kind="ExternalInput").ap()
    bt = nc.dram_tensor("bt", [D, M], f32, kind="ExternalInput").ap()
    cm_o = nc.dram_tensor("cm", [128, NT * NCHUNK], f32,
                          kind="ExternalOutput").ap()
    # rows 0..NT-1: per-tile Pool column partials (cols [WD, M) valid);
    # row NT: colacc final (cols [0, WD) valid)
    colp_o = nc.dram_tensor("colp", [NT + 1, M], f32,
                            kind="ExternalOutput").ap()

    with tile.TileContext(nc) as tc:
        with tc.tile_pool(name="big", bufs=1) as big, \
             tc.tile_pool(name="rows", bufs=2) as rows, \
             tc.tile_pool(name="cps", bufs=2) as cps, \
             tc.tile_pool(name="dmy", bufs=2) as dmy, \
             tc.tile_pool(name="ps", bufs=2, space="PSUM") as ps:
            atb = big.tile([128, HALF], f32r, name="atb")
            btb = big.tile([128, M], f32r, name="btb")
            # tile 0 needs at[:, 0:128] and then bt chunks in matmul order;
            # front-load tiny slices of those so the PE starts ASAP
            nc.sync.dma_start(atb[:, 0:128], at[:, 0:128].bitcast(f32r))
            nc.sync.dma_start(btb[:, 0:512], bt[:, 0:512].bitcast(f32r))
            nc.sync.dma_start(btb[:, 512:1024], bt[:, 512:1024].bitcast(f32r))
            for c in range(1024, M, 1024):
                nc.sync.dma_start(btb[:, c:c + 1024],
                                  bt[:, c:c + 1024].bitcast(f32r))
            nc.sync.dma_start(atb[:, 128:1024], at[:, 128:1024].bitcast(f32r))
            for c in range(1024, HALF, 1024):
                nc.sync.dma_start(atb[:, c:c + 1024],
                                  at[:, c:c + 1024].bitcast(f32r))
            cm_all = big.tile([128, NT * NCHUNK], f32, name="cm_all")
            colacc = big.tile([128, WD], f32, name="colacc")
            for t in range(NT):
                row = rows.tile([128, M], f32, tag="row", name="row")
                for c in range(4):
                    pt = ps.tile([128, 2048], f32, tag="pt", name="pt")
                    for j in range(4):
                        mlo = c * 2048 + j * 512
                        nc.tensor.matmul(pt[:, j * 512:(j + 1) * 512],
                                         atb[:, t * 128:(t + 1) * 128],
                                         btb[:, mlo:mlo + 512],
                                         start=True, stop=True)
                    if t == 0 and c == 0:
                        # 4 narrow copies so the first DVE accum can start
                        # right after the first matmul lands
                        for j in range(4):
                            nc.scalar.copy(row[:, j * 512:(j + 1) * 512],
                                           pt[:, j * 512:(j + 1) * 512])
                    else:
                        nc.scalar.copy(row[:, c * 2048:(c + 1) * 2048], pt[:])
                # row side: chunk maxima via identity tensor_scalar with
                # max-accumulator (2x_2p: all-SBUF operands)
                for c in range(NCHUNK):
                    dummy = dmy.tile([128, CW], f32, tag="dmy", name="dmy")
                    nc.vector.tensor_scalar(
                        dummy[:], row[:, c * CW:(c + 1) * CW], 1.0, None,
                        op0=mybir.AluOpType.mult, op1=mybir.AluOpType.max,
                        accum_out=cm_all[:, t * NCHUNK + c:t * NCHUNK + c + 1])
                # column side, DVE part
                if t == 0:
                    nc.vector.tensor_copy(colacc[:], row[:, 0:WD])
                else:
                    nc.vector.tensor_tensor(colacc[:], colacc[:],
                                            row[:, 0:WD],
                                            op=mybir.AluOpType.max)
                # column side, Pool part: software partition reduce into
                # a separate buffer (keeps rows free of WAR/DMA holds).
                # Last tile: per-2048-chunk so each partial starts right
                # after its ACT copy, shortening the drain.
                if t < 2:
                    for lo, hi in ((WD, 4096), (4096, 6144), (6144, M)):
                        cp = cps.tile([128, hi - lo], f32, tag="cpl",
                                      name="cpe")
                        nc.gpsimd.partition_all_reduce(
                            cp[:], row[:, lo:hi], channels=128,
                            reduce_op=bass_isa.ReduceOp.max)
                        nc.sync.dma_start(colp_o[t:t + 1, lo:hi], cp[0:1, :])
                else:
                    cp = cps.tile([128, M - WD], f32, tag="cp", name="cp")
                    nc.gpsimd.partition_all_reduce(
                        cp[:], row[:, WD:M], channels=128,
                        reduce_op=bass_isa.ReduceOp.max)
                    nc.sync.dma_start(colp_o[t:t + 1, WD:M], cp[0:1, :])
            nc.gpsimd.partition_all_reduce(colacc[:], colacc[:], channels=128,
                                           reduce_op=bass_isa.ReduceOp.max)
            nc.sync.dma_start(colp_o[NT:NT + 1, 0:WD], colacc[0:1, :])
            nc.sync.dma_start(cm_o[:, 0:(NT - 1) * NCHUNK],
                              cm_all[:, 0:(NT - 1) * NCHUNK])
            nc.sync.dma_start(cm_o[:, (NT - 1) * NCHUNK:],
                              cm_all[:, (NT - 1) * NCHUNK:])
    nc.compile()
    return nc


def _build2():
    f32, f32r, u32 = mybir.dt.float32, mybir.dt.float32r, mybir.dt.uint32
    nc = bacc.Bacc("TRN2", target_bir_lowering=False, debug=False,
                   num_devices=NCORES)
    at2 = nc.dram_tensor("at2", [D, GRP * PAD], f32, kind="ExternalInput").ap()
    bt2 = nc.dram_tensor("bt2", [D, M // 2], f32, kind="ExternalInput").ap()
    sg = nc.dram_tensor("sg", [128, NST * 8], f32, kind="ExternalInput").ap()
    idx_o = nc.dram_tensor("idx", [128, NST * 8], u32, kind="ExternalOutput").ap()
    with tile.TileContext(nc) as tc:
        with tc.tile_pool(name="big", bufs=1) as big, \
             tc.tile_pool(name="stg", bufs=4) as stg, \
             tc.tile_pool(name="ps", bufs=4, space="PSUM") as ps:
            a2b = big.tile([128, GRP * PAD], f32r, name="a2b")
            b2b = big.tile([128, M // 2], f32r, name="b2b")
            sgb = big.tile([128, NST * 8], f32, name="sgb")
            nc.sync.dma_start(a2b[:, 0:128], at2[:, 0:128].bitcast(f32r))
            nc.sync.dma_start(b2b[:, 0:512], bt2[:, 0:512].bitcast(f32r))
            nc.sync.dma_start(sgb[:], sg[:])
            # interleave so group 0's matmuls start before all input lands
            na = (GRP * PAD + 1023) // 1024
            nb = (M // 2) // 1024
            for i in range(max(na, nb)):
                if i < na:
                    c = i * 1024
                    lo = 128 if i == 0 else 0
                    w = min(1024, GRP * PAD - c)
                    nc.sync.dma_start(a2b[:, c + lo:c + w],
                                      at2[:, c + lo:c + w].bitcast(f32r))
                if i < nb:
                    c = i * 1024
                    lo = 512 if i == 0 else 0
                    nc.sync.dma_start(b2b[:, c + lo:c + 1024],
                                      bt2[:, c + lo:c + 1024].bitcast(f32r))
            idx8 = big.tile([128, NST * 8], u32, name="idx8")
            KP = PAD // 128
            for g in range(GRP):
                for k in range(KP):
                    st = g * KP + k
                    pt = ps.tile([128, CW], f32, tag="pt", name="pt")
                    nc.tensor.matmul(pt[:],
                                     a2b[:, st * 128:(st + 1) * 128],
                                     b2b[:, g * CW:(g + 1) * CW],
                                     start=True, stop=True)
                    ch = stg.tile([128, CW], f32, tag="ch", name="ch")
                    nc.scalar.copy(ch[:], pt[:])
                    nc.vector.max_index(idx8[:, st * 8:(st + 1) * 8],
                                        sgb[:, st * 8:(st + 1) * 8], ch[:])
            nc.sync.dma_start(idx_o[:, 0:(NST - 1) * 8],
                              idx8[:, 0:(NST - 1) * 8])
            nc.sync.dma_start(idx_o[:, (NST - 1) * 8:],
                              idx8[:, (NST - 1) * 8:])
    nc.compile()
    return nc


_cached = None


def _make_exec(nc):
    import jax
    from jax.sharding import Mesh, PartitionSpec
    from jax.experimental.shard_map import shard_map
    from concourse import bass2jax
    from concourse.bass2jax import _bass_exec_p

    partition_name = nc.partition_id_tensor.name if nc.partition_id_tensor else None
    in_names, out_names, out_avals, out_shapes = [], [], [], []
    for alloc in nc.m.functions[0].allocations:
        if not isinstance(alloc, mybir.MemoryLocationSet):
            continue
        name = alloc.memorylocations[0].name
        if alloc.kind == "ExternalInput":
            if name != partition_name:
                in_names.append(name)
        elif alloc.kind == "ExternalOutput":
            shape = tuple(alloc.tensor_shape)
            dtype = mybir.dt.np(alloc.dtype)
            out_names.append(name)
            out_shapes.append((shape, dtype))
            out_avals.append(jax.core.ShapedArray(shape, dtype))
    n_params = len(in_names)
    n_outs = len(out_names)
    all_in_names = in_names + out_names
    if partition_name is not None:
        all_in_names = all_in_names + [partition_name]

    def _body(*args):
        operands = list(args)
        if partition_name is not None:
            operands.append(bass2jax.partition_id_tensor())
        outs = _bass_exec_p.bind(
            *operands, out_avals=tuple(out_avals), in_names=tuple(all_in_names),
            out_names=tuple(out_names), lowering_input_output_aliases=(),
            sim_require_finite=True, sim_require_nnan=True, nc=nc)
        return tuple(outs)

    devices = jax.devices()[:NCORES]
    mesh = Mesh(np.asarray(devices), ("core",))
    in_specs = (PartitionSpec("core"),) * (n_params + n_outs)
    out_specs = (PartitionSpec("core"),) * n_outs
    fn = jax.jit(shard_map(_body, mesh=mesh, in_specs=in_specs,
                           out_specs=out_specs, check_rep=False),
                 keep_unused=True)
    return {"fn": fn, "in_names": in_names, "out_names": out_names,
            "out_shapes": out_shapes, "nc": nc}


def _run(ex, ins):
    """ins: dict name -> [NCORES, *shape]; returns dict name -> [NCORES, *shape]."""
    concat_in = [np.ascontiguousarray(ins[n].reshape(-1, *ins[n].shape[2:]))
                 for n in ex["in_names"]]
    concat_zeros = [np.zeros((NCORES * s[0], *s[1:]), dt)
                    for (s, dt) in ex["out_shapes"]]
    out_arrs = ex["fn"](*concat_in, *concat_zeros)
    return {name: np.asarray(out_arrs[i]).reshape(NCORES, *ex["out_shapes"][i][0])
            for i, name in enumerate(ex["out_names"])}


def kernel(desc0, desc1):
    global _cached
    desc0 = np.asarray(desc0, dtype=np.float32)
    desc1 = np.asarray(desc1, dtype=np.float32)
    assert desc0.shape == (B, N, D) and desc1.shape == (B, M, D)

    if _cached is None:
        _cached = (_make_exec(_build1()), _make_exec(_build2()))
    ex1, ex2 = _cached

    a_slab = np.stack([desc0[b, h * HALF:(h + 1) * HALF]
                       for b in range(B) for h in range(2)])      # [8,4096,128]
    bt_all = np.stack([desc1[b].transpose(1, 0)
                       for b in range(B) for h in range(2)])      # [8,128,8192]
    at_all = a_slab.transpose(0, 2, 1)                            # [8,128,4096]

    r1 = _run(ex1, {"at": at_all, "bt": bt_all})

    # host glue: score/chunk-argmax + grouping for phase 2
    cm = r1["cm"].reshape(NCORES, 128, NT, NCHUNK).transpose(0, 2, 1, 3) \
                 .reshape(NCORES, HALF, NCHUNK)
    cm_b = cm.reshape(B, N, NCHUNK)
    score0 = cm_b.max(axis=2)                                     # [B, N]
    cstar = cm_b.argmax(axis=2)                                   # [B, N]
    colp = r1["colp"].reshape(B, 2, NT + 1, M)
    colmax = np.empty((B, M), np.float32)
    colmax[:, :WD] = colp[:, :, NT, :WD].max(axis=1)
    colmax[:, WD:] = colp[:, :, :NT, WD:].max(axis=(1, 2))

    at2 = np.zeros((NCORES, D, GRP * PAD), np.float32)
    sg = np.full((NCORES, 128, NST), 1e30, np.float32)
    slot_of_row = np.full((B, N), -1, np.int64)
    core_of_row = np.full((B, N), 0, np.int64)
    overflow = []                                                 # (b, n)
    for b in range(B):
        for g in range(NCHUNK):
            rws = np.nonzero(cstar[b] == g)[0]
            core = 2 * b + (g >= GRP)
            gl = g % GRP                                          # local group
            if len(rws) > PAD:
                overflow.extend((b, n) for n in rws[PAD:])
                rws = rws[:PAD]
            slots = gl * PAD + np.arange(len(rws))
            slot_of_row[b, rws] = slots
            core_of_row[b, rws] = core
            at2[core][:, slots] = desc0[b, rws].T
            sg[core][slots % 128, slots // 128] = score0[b, rws]

    bt2_all = np.stack([desc1[b].T[:, h2 * (M // 2):(h2 + 1) * (M // 2)]
                        for b in range(B) for h2 in range(2)])    # [8,128,4096]
    sg8 = np.repeat(sg, 8, axis=2)                         # [8,128,NST*8]
    r2 = _run(ex2, {"at2": at2, "bt2": bt2_all, "sg": sg8})
    within = r2["idx"][:, :, ::8]                                 # [8, 128, NST]

    sl = np.maximum(slot_of_row, 0)
    cr = core_of_row
    w = within[cr, sl % 128, sl // 128].astype(np.int64)          # [B, N]
    match01 = (cstar * CW + w).astype(np.int32)
    valid = (score0 > 0.1) & \
            (score0 == np.take_along_axis(colmax, match01.astype(np.int64),
                                          axis=1))

    for b, n in overflow:                                         # ~never taken
        simrow = desc0[b, n] @ desc1[b].T
        j = int(simrow.argmax())
        s = simrow.max()
        col = desc0[b] @ desc1[b, j]
        match01[b, n] = j
        score0[b, n] = np.float32(s)
        valid[b, n] = (s > 0.1) & (int(col.argmax()) == n)

    return match01, score0.astype(np.float32), valid
